# revision 27
# baseline (speedup 1.0000x reference)
"""Trainium2 Bass kernel for nn_DependencyTreeModel (dependency-tree matrix-tree loss).

Strategy (data-parallel over batch B=8, one batch element per NeuronCore):
  * Host: permute node 0 to the end (symmetric permutation, det-invariant),
    gather parent rows Qk = h[parent] masked by side (no model FLOPs),
    ship everything bf16 where precision allows.
  * Device, per core, TRANSPOSED space (M = L^T, det-invariant), and
    SIGN-FLIPPED storage (M' = -M; det unchanged since N=1024 is even):
      - biaffine compat^T channels in PSUM via bf16 PE matmuls; exp with
        fused row-sum accumulation; Mrow' = e0+e1 via one bf16 2x TT.
      - blocked LU with grounding G = T + c*11^T (c from pre-Schur diag via
        gpsimd partition_all_reduce), order-1 Neumann + rank-1 det-lemma,
        tr2-only trace-log series.
      - Schur merges RESTRUCTURED for latency: per tile the column-block
        chain C_b is reconstructed TRANSPOSED via PSUM accumulators
        PaccT[b] = sum_kb (W_kb[:,b])^T @ Ct_kb  (stationary = stored W
        slices, so no transposes or copies on the serial chain; chain step
        is one 128-wide matmul + one 128-wide STT).  The wide updates and
        the diag update accumulate in PSUM across kb and are applied with
        a single STT per tile, feeding the W matmuls directly (rinv folded
        into the row-scaled PnTF stationary).
      - loss_b = relu(logdet - gold); host sums: ALPHA * sum(loss_b) / B.
"""
import os
import sys

sys.path.insert(0, "/opt/trn_rl_repo")

import numpy as np

B, N, H = 8, 1024, 256
P = 128
NB = N // P  # 8
HC = H // P  # 2
n2 = N // 2  # 512
ALPHA = 0.25
F32 = np.float32

_CACHE = {}
LAST_RESULTS = None


def _split_multi_waits(bir_bytes, max_waits=1):
    """walrus in this container accepts at most one sync wait per instruction;
    hoist extra waits onto preceding sequencer NoOps (same engine, in order)."""
    import orjson

    d = orjson.loads(bir_bytes)
    for func in d["functions"]:
        for blk in func["blocks"]:
            insts = blk.get("instructions")
            if not insts:
                continue
            new = []
            for ins in insts:
                si = ins.get("sync_info")
                ow = (si or {}).get("on_wait") or []
                if len(ow) > max_waits and ins.get("engine", "Unassigned") != "Unassigned":
                    head, keep = ow[:-max_waits], ow[-max_waits:]
                    for i, w in enumerate(head):
                        nop = {"engine": ins["engine"], "ins": [], "outs": [],
                               "name": f'{ins["name"]}-sw{i}', "opcode": "NoOp",
                               "sync_info": {"on_wait": [w], "on_update": []}}
                        if "debug" in ins:
                            nop["debug"] = ins["debug"]
                        new.append(nop)
                    si["on_wait"] = keep
                new.append(ins)
            blk["instructions"] = new
    return orjson.dumps(d)


def _wchunks(m):
    """W-matmul column chunks: peel 128 head (so the next tile's last merge
    matmuls can start early), then <=512 pieces."""
    out = []
    if m > 128:
        out.append((0, 128))
        o = 128
    else:
        return [(0, m)]
    while o < m:
        w = min(512, m - o)
        out.append((o, w))
        o += w
    return out


# --------------------------------------------------------------------------- #
# device program
# --------------------------------------------------------------------------- #
def build_nc():
    if "nc" in _CACHE:
        return _CACHE["nc"]

    import concourse.bass as bass
    from concourse import bass_isa, library_config
    import concourse.mybir as mybir
    from concourse.bass import MemorySpace, ts
    from concourse.masks import make_identity
    from concourse.tile import TileContext

    dt = mybir.dt.float32
    bf = mybir.dt.bfloat16
    AF = mybir.ActivationFunctionType
    OP = mybir.AluOpType
    AX = mybir.AxisListType
    RED = bass_isa.ReduceOp
    # CoreSim lacks Gelu; swap for Identity under KERNEL_SIM_NOGELU (the
    # numpy sim reference applies the same substitution)
    GELU = AF.Identity if os.environ.get("KERNEL_SIM_NOGELU") else AF.Gelu

    nc = bass.Bass()

    hpT_d = nc.declare_dram_parameter("hpT", [H, N], bf, isOutput=False)
    hN_d = nc.declare_dram_parameter("hN", [N, H], bf, isOutput=False)
    Q0_d = nc.declare_dram_parameter("Q0", [N, H], bf, isOutput=False)
    Q1_d = nc.declare_dram_parameter("Q1", [N, H], bf, isOutput=False)
    W0_d = nc.declare_dram_parameter("W0", [H, H], bf, isOutput=False)
    W1_d = nc.declare_dram_parameter("W1", [H, H], bf, isOutput=False)
    Whd_d = nc.declare_dram_parameter("Whd", [H, 8], bf, isOutput=False)
    bcb_d = nc.declare_dram_parameter("bcb", [2, 4], dt, isOutput=False)
    uu_d = nc.declare_dram_parameter("uu", [P, P], bf, isOutput=False)
    Wr1T_d = nc.declare_dram_parameter("Wr1T", [H, H], bf, isOutput=False)
    br1_d = nc.declare_dram_parameter("br1", [H, 1], dt, isOutput=False)
    Wr2T_d = nc.declare_dram_parameter("Wr2T", [H, 1], bf, isOutput=False)
    bvec_d = nc.declare_dram_parameter("bvec", [1, 4], dt, isOutput=False)
    cm_d = nc.declare_dram_parameter("cm", [2, 4 * N], bf, isOutput=False)
    onehot_d = nc.declare_dram_parameter("onehot", [1, N], dt, isOutput=False)
    cvec_d = nc.declare_dram_parameter("cvec", [P, 3], dt, isOutput=False)
    loss_d = nc.declare_dram_parameter("loss", [1, 1], dt, isOutput=True)

    from contextlib import ExitStack

    with TileContext(nc) as tc, ExitStack() as stack:
        consts = stack.enter_context(tc.tile_pool(name="consts", bufs=1))

        # ---- persistent SBUF tensors; critical-path DMAs first (sync q),
        # late-use bulk (hN/Q0/Q1) on the scalar hwdge queue ----
        hpT = consts.tile([P, HC, N], bf)
        nc.sync.dma_start(hpT, hpT_d.rearrange("(hc p) n -> p hc n", p=P))
        W0s = consts.tile([P, HC, H], bf)
        nc.scalar.dma_start(W0s, W0_d.rearrange("(hc p) g -> p hc g", p=P))
        W1s = consts.tile([P, HC, H], bf)
        nc.scalar.dma_start(W1s, W1_d.rearrange("(hc p) g -> p hc g", p=P))
        Wr1Ts = consts.tile([P, HC, H], bf)
        nc.scalar.dma_start(Wr1Ts, Wr1T_d.rearrange("(hc p) g -> p hc g", p=P))
        Whds = consts.tile([P, HC, 8], bf)
        nc.scalar.dma_start(Whds, Whd_d.rearrange("(hc p) m -> p hc m", p=P))
        bcb = consts.tile([2, 4], dt)
        nc.sync.dma_start(bcb, bcb_d[:, :])
        br1s = consts.tile([P, HC, 1], dt)
        nc.sync.dma_start(br1s, br1_d.rearrange("(hc p) o -> p hc o", p=P))
        Wr2Ts = consts.tile([P, HC, 1], bf)
        nc.scalar.dma_start(Wr2Ts, Wr2T_d.rearrange("(hc p) o -> p hc o", p=P))
        bvecs = consts.tile([1, 4], dt)
        nc.sync.dma_start(bvecs, bvec_d[:, :])
        cvecs = consts.tile([P, 3], dt)
        nc.sync.dma_start(cvecs, cvec_d[:, :])
        uu = consts.tile([P, P], bf)   # ones with row 127 & col 127 zeroed
        nc.sync.dma_start(uu, uu_d[:, :])
        onehot = consts.tile([1, N], dt)
        nc.sync.dma_start(onehot, onehot_d[:, :])
        # bulk, needed late (gold bilinear at it>=2)
        Q0s = consts.tile([P, NB, H], bf)
        nc.sync.dma_start(Q0s, Q0_d.rearrange("(jt p) h -> p jt h", p=P))
        cms = consts.tile([2, 4, N], bf)
        nc.sync.dma_start(cms, cm_d.rearrange("p (g n) -> p g n", g=4))
        hNs = consts.tile([P, NB, H], bf)
        nc.scalar.dma_start(hNs, hN_d.rearrange("(jt p) h -> p jt h", p=P))
        Q1s = consts.tile([P, NB, H], bf)
        nc.scalar.dma_start(Q1s, Q1_d.rearrange("(jt p) h -> p jt h", p=P))

        u_col = cvecs[:, 0:1]          # ones, 0 at 127
        w127n_col = cvecs[:, 2:3]      # -e127

        eye_bf = consts.tile([P, P], bf)
        make_identity(nc, eye_bf)
        eyef = consts.tile([P, P], dt)
        make_identity(nc, eyef)
        # gpsimd ucode with partition_all_reduce (grounding constant c)
        nc.gpsimd.load_library(library_config.attn)
        ones128 = consts.tile([P, P], bf)
        nc.any.memset(ones128, 1.0)
        ones_col = consts.tile([P, 1], dt)
        nc.any.memset(ones_col, 1.0)
        ones_col_bf = consts.tile([P, 1], bf)
        nc.any.memset(ones_col_bf, 1.0)
        ones_row = consts.tile([1, P], dt)
        nc.any.memset(ones_row, 1.0)
        ones_row_bf = consts.tile([1, P], bf)
        nc.any.memset(ones_row_bf, 1.0)
        one11 = consts.tile([1, 1], dt)
        nc.any.memset(one11, 1.0)
        scratch11 = consts.tile([1, 1], dt)
        # warm the gelu table set while input DMAs stream
        nc.scalar.activation(scratch11, one11, GELU)

        UkT = consts.tile([P, 4, N], bf)     # (h W_k)^T, idx = k*2+gt
        Gg = consts.tile([P, HC, N], bf)     # gelu(h W_r1^T + b_r1) transposed
        bcg = consts.tile([2, 4, N], bf)     # pairs: J0 I0 J1 I1
        Mrow = consts.tile([P, NB, N], bf)   # M' = -(L^T): rows = +(e0+e1)
        Wps = consts.tile([P, NB, N], bf)    # per-block W'' = T^-1 Mrow'
        Gbase = consts.tile([P, NB, P], bf)  # c*1 - blk' (pre-merge, per tile)
        rs_sb = consts.tile([1, N], dt)
        exp_rs = consts.tile([1, N], dt)
        erc = consts.tile([P, NB], dt)       # -exp_rs as columns per tile
        cs_neg = consts.tile([P, NB], dt)    # -colsum per tile
        c_cols = consts.tile([P, NB], dt)    # grounding c (orig sign), bcast
        ld_acc = consts.tile([P, 1], dt)
        nc.any.memset(ld_acc, 0.0)
        gold_root = consts.tile([1, 1], dt)
        gdots = consts.tile([2, 4], dt)
        scr2 = consts.tile([2, N], bf)
        sacc = consts.tile([P, 4], dt)       # S_k reduce partials
        scrB = consts.tile([P, P], bf)       # scratch for fused reduces
        scrG = consts.tile([P, P], bf)       # gpsimd-side scratch
        scrS = consts.tile([P, H], bf)
        scrN = consts.tile([1, N], dt)
        rdg = consts.tile([P, 2], dt)        # grounding partials
        rdsum = consts.tile([P, 2], dt)      # after partition all-reduce

        # ================= phase A: weight transforms ====================== #
        with tc.tile_pool(name="paA", bufs=2, space=MemorySpace.PSUM) as paA:
            # U_kT[g, i] = sum_h W_k[h, g] hpT[h, i]
            for k, Wk in ((0, W0s), (1, W1s)):
                for gt in range(HC):
                    ps = paA.tile([P, N], dt, tag="pbig")
                    for ch in range(2):
                        sl = slice(ch * n2, (ch + 1) * n2)
                        for hc in range(HC):
                            nc.tensor.matmul(
                                ps[:, sl], Wk[:, hc, ts(gt, P)], hpT[:, hc, sl],
                                start=(hc == 0), stop=(hc == HC - 1))
                    nc.scalar.copy(UkT[:, k * 2 + gt, 0:n2], ps[:, 0:n2])
                    nc.vector.tensor_copy(UkT[:, k * 2 + gt, n2:N],
                                          ps[:, n2:N])
            # root MLP hidden: Gg = gelu(W_r1 h^T + b_r1)
            for gt in range(HC):
                ps = paA.tile([P, N], dt, tag="pbig")
                for ch in range(2):
                    sl = slice(ch * n2, (ch + 1) * n2)
                    for hc in range(HC):
                        nc.tensor.matmul(
                            ps[:, sl], Wr1Ts[:, hc, ts(gt, P)], hpT[:, hc, sl],
                            start=(hc == 0), stop=(hc == HC - 1))
                nc.scalar.activation(Gg[:, gt, :], ps, GELU, bias=br1s[:, gt, :])

        with (
            tc.tile_pool(name="paS", bufs=1, space=MemorySpace.PSUM) as paS,
            tc.tile_pool(name="ptr0", bufs=2, space=MemorySpace.PSUM) as ptr0,
        ):
            # head/dep pair tiles [2,N]: J_k=[dep_k;1], I_k=[1;head_k+b_k]
            for g in range(4):
                psb = paS.tile([2, N], dt, tag="pbc")
                for ch in range(2):
                    sl = slice(ch * n2, (ch + 1) * n2)
                    for hc in range(HC):
                        nc.tensor.matmul(psb[:, sl],
                                         Whds[:, hc, 2 * g:2 * g + 2],
                                         hpT[:, hc, sl],
                                         start=(hc == 0), stop=(hc == HC - 1))
                nc.vector.tensor_scalar_add(bcg[:, g, :], psb, bcb[:, g:g + 1])
            # root scores row + exp + transposed columns
            psr = paS.tile([2, N], dt, tag="psr")
            for ch in range(2):
                sl = slice(ch * n2, (ch + 1) * n2)
                for gt in range(HC):
                    nc.tensor.matmul(psr[0:1, sl], Wr2Ts[:, gt, :],
                                     Gg[:, gt, sl],
                                     start=(gt == 0), stop=(gt == HC - 1))
            nc.scalar.activation(rs_sb, psr[0:1, :], AF.Identity,
                                 bias=bvecs[:, 2:3])
            nc.scalar.activation(exp_rs, rs_sb, AF.Exp)
            nc.vector.scalar_tensor_tensor(
                out=scrN, in0=onehot, scalar=1.0, in1=rs_sb,
                op0=OP.mult, op1=OP.mult, accum_out=gold_root)
            # gold head/dep/bias dots vs bc pair rows
            for g in range(4):
                nc.vector.scalar_tensor_tensor(
                    out=scr2, in0=cms[:, g, :], scalar=1.0, in1=bcg[:, g, :],
                    op0=OP.mult, op1=OP.mult, accum_out=gdots[:, g:g + 1])
            for it in range(NB):
                tp = ptr0.tile([P, 1], dt, tag="tp")
                nc.tensor.transpose(tp, exp_rs[:, ts(it, P)], one11)
                # negate: Mrow' root column is -exp(rs)
                nc.vector.tensor_scalar_mul(erc[:, it:it + 1], tp, -1.0)

        # ========== main loop: compat tiles + low-latency blocked LU ======= #
        with (
            tc.tile_pool(name="pck", bufs=1, space=MemorySpace.PSUM) as pck,
            tc.tile_pool(name="ppa", bufs=1, space=MemorySpace.PSUM) as ppa,
            tc.tile_pool(name="pdm", bufs=1, space=MemorySpace.PSUM) as pdm,
            tc.tile_pool(name="paw", bufs=1, space=MemorySpace.PSUM) as paw,
            tc.tile_pool(name="pws", bufs=1, space=MemorySpace.PSUM) as pws,
            tc.tile_pool(name="ptr", bufs=1, space=MemorySpace.PSUM) as ptr,
            tc.tile_pool(name="ee", bufs=2) as eep,
            tc.tile_pool(name="fp", bufs=2) as fp,
            tc.tile_pool(name="sp", bufs=3) as sp,
        ):
            # dummy-warmer bank; tail slices host the tiny grounding
            # matmuls (column sums + broadcast)
            dmt = pdm.tile([P, n2], dt, tag="dm")
            for it in range(NB):
                last = it == NB - 1
                mj = P - 1 if last else P
                itsl = ts(it, P)
                m_it = N - it * P          # diag + wide width
                # ---------- compat^T tile it: [128 j, 1024 i] --------------- #
                ck = pck.tile([P, N], dt, tag="ck")
                es1 = eep.tile([P, N], bf, tag="es1")
                r0 = sp.tile([P, 1], dt, tag="r0")
                r1 = sp.tile([P, 1], dt, tag="r1")
                for k in range(2):
                    for ch in range(2):
                        sl = slice(ch * n2, (ch + 1) * n2)
                        for gt in range(HC):
                            nc.tensor.matmul(
                                ck[:, sl], hpT[:, gt, itsl],
                                UkT[:, k * 2 + gt, sl],
                                start=(gt == 0), stop=False)
                        nc.tensor.matmul(
                            ck[:, sl], bcg[:, 2 * k, itsl],
                            bcg[:, 2 * k + 1, sl],
                            start=False, stop=True)
                    with tc.high_priority():
                        if k == 0:
                            nc.scalar.activation(Mrow[:, it, :], ck, AF.Exp,
                                                 accum_out=r0)
                        else:
                            nc.scalar.activation(es1, ck, AF.Exp,
                                                 accum_out=r1)
                # Mrow' = e0 + e1 (bf16 2x TT, in place)
                nc.vector.tensor_add(Mrow[:, it, :], Mrow[:, it, :], es1)
                nc.vector.scalar_tensor_tensor(
                    out=cs_neg[:, it:it + 1], in0=r0, scalar=-1.0, in1=r1,
                    op0=OP.mult, op1=OP.subtract)
                blk = Mrow[:, it, itsl]
                nc.vector.scalar_tensor_tensor(
                    out=blk, in0=eye_bf, scalar=cs_neg[:, it:it + 1], in1=blk,
                    op0=OP.mult, op1=OP.add)
                nc.vector.tensor_copy(Mrow[:, it, N - 1:N], erc[:, it:it + 1])
                # ---------- grounding constant c (from pre-Schur diag) ------ #
                if last:
                    nc.any.memset(rdg, 0.0)
                nc.vector.tensor_reduce(
                    rdg[:mj, 0:1], Mrow[:mj, it, it * P:it * P + mj],
                    AX.X, OP.add)
                nc.vector.scalar_tensor_tensor(
                    out=scrB[:mj, :mj], in0=eye_bf[:mj, :mj], scalar=1.0,
                    in1=blk[:mj, :mj],
                    op0=OP.mult, op1=OP.mult, accum_out=rdg[:mj, 1:2])
                nc.gpsimd.partition_all_reduce(rdsum, rdg, channels=P,
                                               reduce_op=RED.add)
                # c = (sum_diag' - sum_all') * -fac  (primed signs fold in)
                fac = (NB / (NB - it)) / (mj * (mj - 1))
                tcg = sp.tile([P, 1], dt, tag="tcg")
                nc.vector.tensor_sub(tcg, rdsum[:, 1:2], rdsum[:, 0:1])
                nc.vector.tensor_scalar_mul(c_cols[:, it:it + 1], tcg, -fac)
                if not last:
                    # Gbase = c*1 - blk' (pre-merge part of grounded diag)
                    nc.vector.scalar_tensor_tensor(
                        out=Gbase[:, it, :], in0=ones128,
                        scalar=c_cols[:, it:it + 1], in1=blk,
                        op0=OP.mult, op1=OP.subtract)
                # ---------- gold bilinear PE filler (needs hN/Q DMAs) ------- #
                if 2 <= it <= 5:
                    g = it - 2
                    kq, ht = g // 2, g % 2
                    Qs = Q0s if kq == 0 else Q1s
                    Ws = W0s if kq == 0 else W1s
                    stile = pws.tile([P, n2], dt, tag="ws")
                    spp = stile[:, 0:H]
                    for jt in range(NB):
                        nc.tensor.matmul(spp, Qs[:, jt, ts(ht, P)],
                                         hNs[:, jt, :],
                                         start=(jt == 0), stop=(jt == NB - 1))
                    nc.vector.scalar_tensor_tensor(
                        out=scrS, in0=Ws[:, ht, :], scalar=1.0, in1=spp,
                        op0=OP.mult, op1=OP.mult,
                        accum_out=sacc[:, g:g + 1])

                # ---------- At transposes (base C blocks, off-path) --------- #
                Att = None
                if it >= 1:
                    Att = fp.tile([P, 7, P], bf, tag="At")
                    for b in range(it):
                        tps = ptr.tile([P, P], bf, tag="tr")
                        nc.tensor.transpose(tps, Mrow[:, it, ts(b, P)], eye_bf)
                        if b % 2 == 0:
                            nc.scalar.copy(Att[:, b, :], tps)
                        else:
                            nc.vector.tensor_copy(Att[:, b, :], tps)

                # ---------- merge chain (transposed-C reconstruction) ------- #
                # Ct_b persists in SBUF, so slot b's accumulation
                # sum_kb<b (W_kb[:,b])^T Ct_kb runs as ONE consecutive PSUM
                # group right before its consumption: group opens and closes
                # within a step (PSUM groups can't be read mid-accumulation),
                # alternating between two banks.  Only the group's LAST
                # matmul is on the serial chain.
                accW = None
                if it >= 1:
                    accW = paw.tile([P, n2], dt, tag="aw")
                    pacc = None
                    if it >= 2:
                        pacc = ppa.tile([P, N], dt, tag="pacc")
                    Ctt = fp.tile([P, 7, P], bf, tag="Ct")
                    # first pass covers diag+wide up to one bank; the tail
                    # (it<=3) runs as a second pass through the same slot
                    wch = [(0, min(m_it, n2))]
                    for b in range(it):
                        if b == 0:
                            Ct = Att[:, 0, :]
                        else:
                            # slot group for this step: all kb<b terms;
                            # slots ping-pong between the two banks so a
                            # group can open with old terms while the
                            # previous slot (other bank) is being read.
                            # Only the LAST matmul is on the serial chain.
                            slot = pacc[:, (b % 2) * n2:(b % 2) * n2 + P]
                            for kb in range(b):
                                nc.tensor.matmul(
                                    slot, Wps[:, kb, ts(b, P)],
                                    (Att[:, 0, :] if kb == 0
                                     else Ctt[:, kb, :]),
                                    start=(kb == 0), stop=(kb == b - 1))
                            Ct = Ctt[:, b, :]
                            with tc.high_priority():
                                nc.vector.scalar_tensor_tensor(
                                    out=Ct, in0=slot, scalar=1.0,
                                    in1=Att[:, b, :],
                                    op0=OP.mult, op1=OP.add)
                        # diag+wide accumulation (one matmul per bank per kb)
                        for (o, wd) in wch:
                            nc.tensor.matmul(
                                accW[:, o:o + wd], Ct,
                                Wps[:, b, it * P + o:it * P + o + wd],
                                start=(b == 0), stop=(b == it - 1))

                # ---------- elimination of block it ------------------------- #
                if last:
                    # merged diag block, bordered handling
                    blk7 = fp.tile([P, P], bf, tag="blk7")
                    nc.vector.scalar_tensor_tensor(
                        out=blk7, in0=accW[:, 0:P], scalar=1.0, in1=blk,
                        op0=OP.mult, op1=OP.add)
                    smt = pws.tile([P, n2], dt, tag="ws")
                    rtp = smt[:, 0:1]
                    nc.tensor.matmul(rtp, blk7, eye_bf[:, P - 1:P],
                                     start=True, stop=True)
                    rvec = sp.tile([P, 1], dt, tag="rvec")
                    nc.vector.tensor_scalar_mul(rvec, rtp, -1.0)
                    cvec_sb = sp.tile([P, 1], dt, tag="cvec")
                    nc.vector.tensor_scalar_mul(cvec_sb, blk7[:, P - 1:P], -1.0)
                    nc.vector.tensor_copy(blk7[:, P - 1:P], w127n_col)
                    G = fp.tile([P, P], bf, tag="G")
                    nc.vector.scalar_tensor_tensor(
                        out=G, in0=uu, scalar=c_cols[:, it:it + 1], in1=blk7,
                        op0=OP.mult, op1=OP.subtract)
                elif it == 0:
                    G = Gbase[:, 0, :]
                else:
                    G = fp.tile([P, P], bf, tag="G")
                    with tc.high_priority():
                        nc.vector.scalar_tensor_tensor(
                            out=G, in0=accW[:, 0:P], scalar=-1.0,
                            in1=Gbase[:, it, :], op0=OP.mult, op1=OP.add)
                with tc.high_priority():
                    d = sp.tile([P, 1], dt, tag="d")
                    nc.vector.scalar_tensor_tensor(
                        out=scrB, in0=eye_bf, scalar=1.0, in1=G,
                        op0=OP.mult, op1=OP.mult, accum_out=d)
                    rinv = sp.tile([P, 1], dt, tag="rinv")
                    nc.vector.reciprocal(rinv, d)
                    F = fp.tile([P, P], bf, tag="F")
                    nc.vector.scalar_tensor_tensor(
                        out=F, in0=G, scalar=rinv, in1=eye_bf,
                        op0=OP.mult, op1=OP.subtract)
                    tps = ptr.tile([P, P], bf, tag="tr")
                    nc.tensor.transpose(tps, F, eye_bf)
                lnd = sp.tile([P, 1], dt, tag="lnd")
                nc.scalar.activation(lnd, d, AF.Ln)
                nc.vector.tensor_add(ld_acc, ld_acc, lnd)
                # merged off-diag row MB' = base + acc (fills the transpose
                # wait on DVE); wide tail (it<=3) via a second one-bank pass
                MBv = None
                if not last:
                    m = m_it - P
                    if it == 0:
                        MBv = Mrow[:, 0, P:N]
                    else:
                        MB = eep.tile([P, 768], bf, tag="MB")
                        mA = min(m_it, n2) - P
                        nc.vector.scalar_tensor_tensor(
                            out=MB[:, 0:mA], in0=accW[:, P:P + mA],
                            scalar=1.0,
                            in1=Mrow[:, it, (it + 1) * P:(it + 1) * P + mA],
                            op0=OP.mult, op1=OP.add)
                        if m_it > n2:
                            mB = m_it - n2
                            accB = paw.tile([P, n2], dt, tag="aw")
                            for b2 in range(it):
                                nc.tensor.matmul(
                                    accB[:, 0:mB],
                                    (Att[:, 0, :] if b2 == 0
                                     else Ctt[:, b2, :]),
                                    Wps[:, b2, it * P + n2:N],
                                    start=(b2 == 0), stop=(b2 == it - 1))
                            nc.vector.scalar_tensor_tensor(
                                out=MB[:, mA:m], in0=accB[:, 0:mB],
                                scalar=1.0,
                                in1=Mrow[:, it, it * P + n2:N],
                                op0=OP.mult, op1=OP.add)
                        MBv = MB[:, 0:m]
                Ft = fp.tile([P, P], bf, tag="Ft")
                with tc.high_priority():
                    nc.scalar.copy(Ft, tps)
                t2 = sp.tile([P, 1], dt, tag="t2")
                nc.vector.scalar_tensor_tensor(
                    out=scrB, in0=F, scalar=-0.5, in1=Ft,
                    op0=OP.mult, op1=OP.mult, accum_out=t2)
                nc.vector.tensor_add(ld_acc, ld_acc, t2)

                if not last:
                    # critical W path: order-1 Neumann WITHOUT the rank-1
                    # det-lemma W-correction (logdet keeps its correction
                    # below; the W impact is higher-order and within the
                    # error budget) -> chain is G,d,rinv,F,Ft,PnTFs,W only.
                    with tc.high_priority():
                        PnTF = fp.tile([P, P], bf, tag="PnTF")
                        nc.vector.scalar_tensor_tensor(
                            out=PnTF, in0=Ft, scalar=-1.0, in1=eye_bf,
                            op0=OP.mult, op1=OP.add)
                        PnTFs = fp.tile([P, P], bf, tag="PnTFs")
                        nc.vector.tensor_scalar_mul(PnTFs, PnTF, rinv)
                        m = m_it - P
                        wps = pws.tile([P, n2], dt, tag="ws")
                        nc.tensor.matmul(wps[:, 0:P], PnTFs, MBv[:, 0:P],
                                         start=True, stop=True)
                        nc.scalar.copy(Wps[:, it, (it + 1) * P:
                                           (it + 2) * P], wps[:, 0:P])
                    for (o, wd) in _wchunks(m)[1:]:
                        wps = pws.tile([P, n2], dt, tag="ws")
                        nc.tensor.matmul(wps[:, :wd], PnTFs, MBv[:, o:o + wd],
                                         start=True, stop=True)
                        nc.scalar.copy(Wps[:, it, (it + 1) * P + o:
                                           (it + 1) * P + o + wd],
                                       wps[:, :wd])
                    # det-lemma logdet scalar correction dropped for
                    # interior blocks: the grounding shift contributes
                    # ~1e-4 relative, far under the 2e-2 budget
                else:
                    # bordered last block, root in column 127
                    smt2 = pws.tile([P, n2], dt, tag="ws")
                    F2p = smt2[:, 0:P]
                    nc.tensor.matmul(F2p, F, Ft, start=True, stop=True)
                    PnT2 = fp.tile([P, P], bf, tag="PnT2")
                    nc.vector.scalar_tensor_tensor(
                        out=PnT2, in0=Ft, scalar=-1.0, in1=eye_bf,
                        op0=OP.mult, op1=OP.add)
                    nc.vector.tensor_add(PnT2, PnT2, F2p)
                    x0 = sp.tile([P, 1], dt, tag="x0")
                    nc.vector.tensor_mul(x0, rinv, u_col)
                    x0_bf = sp.tile([P, 1], bf, tag="x0b")
                    nc.vector.tensor_copy(x0_bf, x0)
                    qp = smt2[:, P:P + 1]
                    nc.tensor.matmul(qp, PnT2, x0_bf, start=True, stop=True)
                    qm = sp.tile([P, 1], dt, tag="qm")
                    nc.vector.tensor_mul(qm, qp, u_col)
                    chat = sp.tile([P, 1], dt, tag="chat")
                    nc.vector.tensor_mul(chat, cvec_sb, u_col)
                    x0c = sp.tile([P, 1], dt, tag="x0c")
                    nc.vector.tensor_mul(x0c, rinv, chat)
                    x0c_bf = sp.tile([P, 1], bf, tag="x0cb")
                    nc.vector.tensor_copy(x0c_bf, x0c)
                    y1p = smt2[:, P + 1:P + 2]
                    nc.tensor.matmul(y1p, PnT2, x0c_bf, start=True, stop=True)
                    y1m = sp.tile([P, 1], dt, tag="y1m")
                    nc.vector.tensor_mul(y1m, y1p, u_col)
                    dots = smt2[0:1, 2 * P:2 * P + 5]
                    nc.tensor.matmul(dots[:, 0:1], y1m, ones_col,
                                     start=True, stop=True)
                    nc.tensor.matmul(dots[:, 1:2], rvec, y1m,
                                     start=True, stop=True)
                    nc.tensor.matmul(dots[:, 2:3], rvec, qm,
                                     start=True, stop=True)
                    nc.tensor.matmul(dots[:, 3:4], qm, ones_col,
                                     start=True, stop=True)
                    nc.tensor.matmul(dots[:, 4:5], rvec, eyef[:, P - 1:P],
                                     start=True, stop=True)
                    dsb = sp.tile([1, 5], dt, tag="dsb")
                    nc.vector.tensor_copy(dsb, dots)
                    tac = sp.tile([1, 1], dt, tag="tac")
                    nc.vector.tensor_mul(tac, dsb[:, 3:4],
                                         c_cols[0:1, it:it + 1])
                    detr = sp.tile([1, 1], dt, tag="detr")
                    nc.vector.tensor_scalar(
                        out=detr, in0=tac, scalar1=-1.0, scalar2=1.0,
                        op0=OP.mult, op1=OP.add)
                    lndr = sp.tile([1, 1], dt, tag="lndr")
                    nc.scalar.activation(lndr, detr, AF.Ln)
                    nc.vector.tensor_add(ld_acc[0:1, :], ld_acc[0:1, :], lndr)
                    invdr = sp.tile([1, 1], dt, tag="invdr")
                    nc.vector.reciprocal(invdr, detr)
                    gam = sp.tile([1, 1], dt, tag="gam")
                    nc.vector.tensor_mul(gam, c_cols[0:1, it:it + 1], invdr)
                    bg = sp.tile([1, 1], dt, tag="bg")
                    nc.vector.tensor_mul(bg, dsb[:, 0:1], gam)
                    t3 = sp.tile([1, 1], dt, tag="t3")
                    nc.vector.tensor_mul(t3, bg, dsb[:, 2:3])
                    t4 = sp.tile([1, 1], dt, tag="t4")
                    nc.vector.tensor_sub(t4, dsb[:, 4:5], dsb[:, 1:2])
                    sca = sp.tile([1, 1], dt, tag="sca")
                    nc.vector.tensor_sub(sca, t4, t3)
                    lnsc = sp.tile([1, 1], dt, tag="lnsc")
                    nc.scalar.activation(lnsc, sca, AF.Ln)
                    nc.vector.tensor_add(ld_acc[0:1, :], ld_acc[0:1, :], lnsc)

            # HAM warm-keeper: lowest-priority dummy matmuls that the
            # scheduler slots into PE idle gaps, keeping the activity
            # monitor from re-throttling the PE clock to 1.2 GHz.  Each
            # batch reads its iteration's Mrow tile so late batches cannot
            # be hoisted early (supply survives into the tail).
            for it, ndm in enumerate((15, 10, 15, 15, 25, 25, 30, 35)):
                for _ in range(ndm):
                    nc.tensor.matmul(dmt[:, 0:P], eye_bf, Mrow[:, it, 0:P],
                                     start=True, stop=True)

        # ================= gold bilinear + finale ========================== #
        with (
            tc.tile_pool(name="pf", bufs=1, space=MemorySpace.PSUM) as pf,
            tc.tile_pool(name="fin", bufs=1) as finp,
        ):
            sg = finp.tile([P, 1], dt, tag="sg")
            nc.vector.tensor_add(sg, sacc[:, 0:1], sacc[:, 1:2])
            nc.vector.tensor_add(sg, sg, sacc[:, 2:3])
            nc.vector.tensor_add(sg, sg, sacc[:, 3:4])
            fin = pf.tile([1, 8], dt, tag="fin")
            nc.tensor.matmul(fin[:, 0:1], ld_acc, ones_col, start=True, stop=True)
            nc.tensor.matmul(fin[:, 1:2], sg, ones_col, start=True, stop=True)
            gsum = finp.tile([2, 1], dt, tag="gsum")
            nc.vector.tensor_reduce(gsum, gdots, AX.X, OP.add)
            nc.tensor.matmul(fin[:, 2:3], gsum, ones_col[0:2, :],
                             start=True, stop=True)
            fsb = finp.tile([1, 3], dt, tag="fsb")
            nc.vector.tensor_copy(fsb, fin[:, 0:3])
            f1 = finp.tile([1, 1], dt, tag="f1")
            nc.vector.tensor_sub(f1, fsb[:, 0:1], fsb[:, 1:2])
            f2 = finp.tile([1, 1], dt, tag="f2")
            nc.vector.tensor_sub(f2, f1, fsb[:, 2:3])
            f3 = finp.tile([1, 1], dt, tag="f3")
            nc.vector.tensor_sub(f3, f2, gold_root)
            out_sb = finp.tile([1, 1], dt, tag="out")
            nc.scalar.activation(out_sb, f3, AF.Relu)
            nc.sync.dma_start(loss_d[:, :], out_sb)

    _CACHE["nc"] = nc
    return nc


def finalize_nc(nc):
    """Prepare nc for NEFF compilation (mutates the module; sim-incompatible)."""
    if getattr(nc, "_finalized", False):
        return nc
    from concourse import mybir

    mybir.codegen_inst_isa_subclasses(nc)
    fixed_json = _split_multi_waits(nc.to_json_bytes())
    nc.to_json_bytes = lambda: fixed_json
    nc._finalized = True
    return nc


# --------------------------------------------------------------------------- #
# host-side sharding / prep
# --------------------------------------------------------------------------- #
def _cvec():
    c = np.zeros((P, 3), F32)
    c[:, 0] = 1.0
    c[P - 1, 0] = 0.0
    c[P - 1, 1] = 1.0
    c[P - 1, 2] = -1.0
    return c


def prep_in_maps(inputs):
    import ml_dtypes

    BF = ml_dtypes.bfloat16
    h_cat = np.asarray(inputs["h_cat"], F32)
    left = np.asarray(inputs["left_adj"], F32)
    right = np.asarray(inputs["right_adj"], F32)
    roots = np.asarray(inputs["roots"])
    W_bilin = np.asarray(inputs["W_bilin"], F32)
    b_bilin = np.asarray(inputs["b_bilin"], F32)
    W_head = np.asarray(inputs["W_head"], F32)
    W_dep = np.asarray(inputs["W_dep"], F32)
    W_r1 = np.asarray(inputs["W_r1"], F32)
    b_r1 = np.asarray(inputs["b_r1"], F32)
    W_r2 = np.asarray(inputs["W_r2"], F32)
    b_r2 = np.asarray(inputs["b_r2"], F32)

    # Whd col pairs -> [dep0,0], [0,head0], [dep1,0], [0,head1]
    z = np.zeros(H, F32)
    whd = np.stack([W_dep[0], z, z, W_head[0], W_dep[1], z, z, W_head[1]],
                   axis=1)
    bcb = np.array([[0.0, 1.0, 0.0, 1.0],
                    [1.0, b_bilin[0], 1.0, b_bilin[1]]], F32)
    uu = np.ones((P, P), F32)
    uu[:, P - 1] = 0.0
    uu[P - 1, :] = 0.0
    shared = {
        "W0": np.ascontiguousarray(W_bilin[0]).astype(BF),
        "W1": np.ascontiguousarray(W_bilin[1]).astype(BF),
        "Whd": np.ascontiguousarray(whd).astype(BF),
        "bcb": bcb,
        "uu": uu.astype(BF),
        "Wr1T": np.ascontiguousarray(W_r1.T).astype(BF),
        "br1": np.ascontiguousarray(b_r1.reshape(H, 1)),
        "Wr2T": np.ascontiguousarray(W_r2.reshape(1, H).T).astype(BF),
        "bvec": np.ascontiguousarray(
            np.array([b_bilin[0], b_bilin[1], b_r2.reshape(-1)[0], 0.0],
                     F32).reshape(1, 4)),
        "cvec": _cvec(),
    }
    in_maps = []
    idx = np.arange(N)
    for b in range(B):
        hp = np.roll(h_cat[b], -1, axis=0)
        Lp = np.roll(np.roll(left[b], -1, axis=0), -1, axis=1)
        Rp = np.roll(np.roll(right[b], -1, axis=0), -1, axis=1)
        par = np.argmax(Lp + Rp, axis=0)
        mask0 = Lp[par, idx] > 0
        mask1 = Rp[par, idx] > 0
        Q0 = np.where(mask0[:, None], hp[par], 0).astype(F32)
        Q1 = np.where(mask1[:, None], hp[par], 0).astype(F32)
        # cm pairs: J_k -> [mask_k; 0], I_k -> [0; cnt_k]
        cm = np.zeros((2, 4, N), F32)
        cm[0, 0] = mask0.astype(F32)
        cm[1, 1] = np.bincount(par[mask0], minlength=N)
        cm[0, 2] = mask1.astype(F32)
        cm[1, 3] = np.bincount(par[mask1], minlength=N)
        cm = cm.reshape(2, 4 * N)
        onehot = np.zeros((1, N), F32)
        onehot[0, (int(roots[b]) - 1) % N] = 1.0
        m = dict(shared)
        m["hpT"] = np.ascontiguousarray(hp.T).astype(BF)
        m["hN"] = np.ascontiguousarray(hp).astype(BF)
        m["Q0"] = np.ascontiguousarray(Q0).astype(BF)
        m["Q1"] = np.ascontiguousarray(Q1).astype(BF)
        m["cm"] = np.ascontiguousarray(cm).astype(BF)
        m["onehot"] = onehot
        in_maps.append(m)
    return in_maps


def kernel(**inputs):
    global LAST_RESULTS
    nc = finalize_nc(build_nc())
    in_maps = prep_in_maps(inputs)
    from concourse.bass_utils import run_bass_kernel_spmd

    trace = bool(os.environ.get("KERNEL_TRACE"))
    res = run_bass_kernel_spmd(nc, in_maps, list(range(B)), trace=trace)
    LAST_RESULTS = res
    losses = np.array([res.results[i]["loss"][0, 0] for i in range(B)], F32)
    return np.asarray(F32(ALPHA) * losses.sum(dtype=F32) / F32(B))


# revision 28
# speedup vs baseline: 1.0091x; 1.0091x over previous
"""Trainium2 Bass kernel for nn_DependencyTreeModel (dependency-tree matrix-tree loss).

Strategy (data-parallel over batch B=8, one batch element per NeuronCore):
  * Host: permute node 0 to the end (symmetric permutation, det-invariant),
    gather parent rows Qk = h[parent] masked by side (no model FLOPs),
    ship everything bf16 where precision allows.
  * Device, per core, TRANSPOSED space (M = L^T, det-invariant), and
    SIGN-FLIPPED storage (M' = -M; det unchanged since N=1024 is even):
      - biaffine compat^T channels in PSUM via bf16 PE matmuls; exp with
        fused row-sum accumulation; Mrow' = e0+e1 via one bf16 2x TT.
      - blocked LU with grounding G = T + c*11^T (c from pre-Schur diag via
        gpsimd partition_all_reduce), order-1 Neumann + rank-1 det-lemma,
        tr2-only trace-log series.
      - Schur merges RESTRUCTURED for latency: per tile the column-block
        chain C_b is reconstructed TRANSPOSED via PSUM accumulators
        PaccT[b] = sum_kb (W_kb[:,b])^T @ Ct_kb  (stationary = stored W
        slices, so no transposes or copies on the serial chain; chain step
        is one 128-wide matmul + one 128-wide STT).  The wide updates and
        the diag update accumulate in PSUM across kb and are applied with
        a single STT per tile, feeding the W matmuls directly (rinv folded
        into the row-scaled PnTF stationary).
      - loss_b = relu(logdet - gold); host sums: ALPHA * sum(loss_b) / B.
"""
import os
import sys

sys.path.insert(0, "/opt/trn_rl_repo")

import numpy as np

B, N, H = 8, 1024, 256
P = 128
NB = N // P  # 8
HC = H // P  # 2
n2 = N // 2  # 512
ALPHA = 0.25
F32 = np.float32

_CACHE = {}
LAST_RESULTS = None


def _split_multi_waits(bir_bytes, max_waits=1):
    """walrus in this container accepts at most one sync wait per instruction;
    hoist extra waits onto preceding sequencer NoOps (same engine, in order)."""
    import orjson

    d = orjson.loads(bir_bytes)
    for func in d["functions"]:
        for blk in func["blocks"]:
            insts = blk.get("instructions")
            if not insts:
                continue
            new = []
            for ins in insts:
                si = ins.get("sync_info")
                ow = (si or {}).get("on_wait") or []
                if len(ow) > max_waits and ins.get("engine", "Unassigned") != "Unassigned":
                    head, keep = ow[:-max_waits], ow[-max_waits:]
                    for i, w in enumerate(head):
                        nop = {"engine": ins["engine"], "ins": [], "outs": [],
                               "name": f'{ins["name"]}-sw{i}', "opcode": "NoOp",
                               "sync_info": {"on_wait": [w], "on_update": []}}
                        if "debug" in ins:
                            nop["debug"] = ins["debug"]
                        new.append(nop)
                    si["on_wait"] = keep
                new.append(ins)
            blk["instructions"] = new
    return orjson.dumps(d)


def _wchunks(m):
    """W-matmul column chunks: peel 128 head (so the next tile's last merge
    matmuls can start early), then <=512 pieces."""
    out = []
    if m > 128:
        out.append((0, 128))
        o = 128
    else:
        return [(0, m)]
    while o < m:
        w = min(512, m - o)
        out.append((o, w))
        o += w
    return out


# --------------------------------------------------------------------------- #
# device program
# --------------------------------------------------------------------------- #
def build_nc():
    if "nc" in _CACHE:
        return _CACHE["nc"]

    import concourse.bass as bass
    from concourse import bass_isa, library_config
    import concourse.mybir as mybir
    from concourse.bass import MemorySpace, ts
    from concourse.masks import make_identity
    from concourse.tile import TileContext

    dt = mybir.dt.float32
    bf = mybir.dt.bfloat16
    AF = mybir.ActivationFunctionType
    OP = mybir.AluOpType
    AX = mybir.AxisListType
    RED = bass_isa.ReduceOp
    # CoreSim lacks Gelu; swap for Identity under KERNEL_SIM_NOGELU (the
    # numpy sim reference applies the same substitution)
    GELU = AF.Identity if os.environ.get("KERNEL_SIM_NOGELU") else AF.Gelu

    nc = bass.Bass()

    hpT_d = nc.declare_dram_parameter("hpT", [H, N], bf, isOutput=False)
    hN_d = nc.declare_dram_parameter("hN", [N, H], bf, isOutput=False)
    Q0_d = nc.declare_dram_parameter("Q0", [N, H], bf, isOutput=False)
    Q1_d = nc.declare_dram_parameter("Q1", [N, H], bf, isOutput=False)
    W0_d = nc.declare_dram_parameter("W0", [H, H], bf, isOutput=False)
    W1_d = nc.declare_dram_parameter("W1", [H, H], bf, isOutput=False)
    Whd_d = nc.declare_dram_parameter("Whd", [H, 8], bf, isOutput=False)
    bcb_d = nc.declare_dram_parameter("bcb", [2, 4], dt, isOutput=False)
    uu_d = nc.declare_dram_parameter("uu", [P, P], bf, isOutput=False)
    Wr1T_d = nc.declare_dram_parameter("Wr1T", [H, H], bf, isOutput=False)
    br1_d = nc.declare_dram_parameter("br1", [H, 1], dt, isOutput=False)
    Wr2T_d = nc.declare_dram_parameter("Wr2T", [H, 1], bf, isOutput=False)
    bvec_d = nc.declare_dram_parameter("bvec", [1, 4], dt, isOutput=False)
    cm_d = nc.declare_dram_parameter("cm", [2, 4 * N], bf, isOutput=False)
    onehot_d = nc.declare_dram_parameter("onehot", [1, N], dt, isOutput=False)
    cvec_d = nc.declare_dram_parameter("cvec", [P, 3], dt, isOutput=False)
    loss_d = nc.declare_dram_parameter("loss", [1, 1], dt, isOutput=True)

    from contextlib import ExitStack

    with TileContext(nc) as tc, ExitStack() as stack:
        consts = stack.enter_context(tc.tile_pool(name="consts", bufs=1))

        # ---- persistent SBUF tensors; critical-path DMAs first (sync q),
        # late-use bulk (hN/Q0/Q1) on the scalar hwdge queue ----
        hpT = consts.tile([P, HC, N], bf)
        nc.sync.dma_start(hpT, hpT_d.rearrange("(hc p) n -> p hc n", p=P))
        W0s = consts.tile([P, HC, H], bf)
        nc.scalar.dma_start(W0s, W0_d.rearrange("(hc p) g -> p hc g", p=P))
        W1s = consts.tile([P, HC, H], bf)
        nc.scalar.dma_start(W1s, W1_d.rearrange("(hc p) g -> p hc g", p=P))
        Wr1Ts = consts.tile([P, HC, H], bf)
        nc.scalar.dma_start(Wr1Ts, Wr1T_d.rearrange("(hc p) g -> p hc g", p=P))
        Whds = consts.tile([P, HC, 8], bf)
        nc.scalar.dma_start(Whds, Whd_d.rearrange("(hc p) m -> p hc m", p=P))
        bcb = consts.tile([2, 4], dt)
        nc.sync.dma_start(bcb, bcb_d[:, :])
        br1s = consts.tile([P, HC, 1], dt)
        nc.sync.dma_start(br1s, br1_d.rearrange("(hc p) o -> p hc o", p=P))
        Wr2Ts = consts.tile([P, HC, 1], bf)
        nc.scalar.dma_start(Wr2Ts, Wr2T_d.rearrange("(hc p) o -> p hc o", p=P))
        bvecs = consts.tile([1, 4], dt)
        nc.sync.dma_start(bvecs, bvec_d[:, :])
        cvecs = consts.tile([P, 3], dt)
        nc.sync.dma_start(cvecs, cvec_d[:, :])
        uu = consts.tile([P, P], bf)   # ones with row 127 & col 127 zeroed
        nc.sync.dma_start(uu, uu_d[:, :])
        onehot = consts.tile([1, N], dt)
        nc.sync.dma_start(onehot, onehot_d[:, :])
        # bulk, needed late (gold bilinear at it>=2)
        Q0s = consts.tile([P, NB, H], bf)
        nc.sync.dma_start(Q0s, Q0_d.rearrange("(jt p) h -> p jt h", p=P))
        cms = consts.tile([2, 4, N], bf)
        nc.sync.dma_start(cms, cm_d.rearrange("p (g n) -> p g n", g=4))
        hNs = consts.tile([P, NB, H], bf)
        nc.scalar.dma_start(hNs, hN_d.rearrange("(jt p) h -> p jt h", p=P))
        Q1s = consts.tile([P, NB, H], bf)
        nc.scalar.dma_start(Q1s, Q1_d.rearrange("(jt p) h -> p jt h", p=P))

        u_col = cvecs[:, 0:1]          # ones, 0 at 127
        w127n_col = cvecs[:, 2:3]      # -e127

        eye_bf = consts.tile([P, P], bf)
        make_identity(nc, eye_bf)
        eyef = consts.tile([P, P], dt)
        make_identity(nc, eyef)
        # gpsimd ucode with partition_all_reduce (grounding constant c)
        nc.gpsimd.load_library(library_config.attn)
        ones128 = consts.tile([P, P], bf)
        nc.any.memset(ones128, 1.0)
        ones_col = consts.tile([P, 1], dt)
        nc.any.memset(ones_col, 1.0)
        ones_col_bf = consts.tile([P, 1], bf)
        nc.any.memset(ones_col_bf, 1.0)
        ones_row = consts.tile([1, P], dt)
        nc.any.memset(ones_row, 1.0)
        ones_row_bf = consts.tile([1, P], bf)
        nc.any.memset(ones_row_bf, 1.0)
        one11 = consts.tile([1, 1], dt)
        nc.any.memset(one11, 1.0)
        scratch11 = consts.tile([1, 1], dt)
        # warm the gelu table set while input DMAs stream
        nc.scalar.activation(scratch11, one11, GELU)

        UkT = consts.tile([P, 4, N], bf)     # (h W_k)^T, idx = k*2+gt
        Gg = consts.tile([P, HC, N], bf)     # gelu(h W_r1^T + b_r1) transposed
        bcg = consts.tile([2, 4, N], bf)     # pairs: J0 I0 J1 I1
        Mrow = consts.tile([P, NB, N], bf)   # M' = -(L^T): rows = +(e0+e1)
        Wps = consts.tile([P, NB, N], bf)    # per-block W'' = T^-1 Mrow'
        Gbase = consts.tile([P, NB, P], bf)  # c*1 - blk' (pre-merge, per tile)
        rs_sb = consts.tile([1, N], dt)
        exp_rs = consts.tile([1, N], dt)
        erc = consts.tile([P, NB], dt)       # -exp_rs as columns per tile
        cs_neg = consts.tile([P, NB], dt)    # -colsum per tile
        c_cols = consts.tile([P, NB], dt)    # grounding c (orig sign), bcast
        ld_acc = consts.tile([P, 1], dt)
        nc.any.memset(ld_acc, 0.0)
        gold_root = consts.tile([1, 1], dt)
        gdots = consts.tile([2, 4], dt)
        scr2 = consts.tile([2, N], bf)
        sacc = consts.tile([P, 4], dt)       # S_k reduce partials
        scrB = consts.tile([P, P], bf)       # scratch for fused reduces
        scrG = consts.tile([P, P], bf)       # gpsimd-side scratch
        scrS = consts.tile([P, H], bf)
        scrN = consts.tile([1, N], dt)
        rdg = consts.tile([P, 2], dt)        # grounding partials
        rdsum = consts.tile([P, 2], dt)      # after partition all-reduce

        # ================= phase A: weight transforms ====================== #
        with tc.tile_pool(name="paA", bufs=2, space=MemorySpace.PSUM) as paA:
            # U_kT[g, i] = sum_h W_k[h, g] hpT[h, i]
            for k, Wk in ((0, W0s), (1, W1s)):
                for gt in range(HC):
                    ps = paA.tile([P, N], dt, tag="pbig")
                    for ch in range(2):
                        sl = slice(ch * n2, (ch + 1) * n2)
                        for hc in range(HC):
                            nc.tensor.matmul(
                                ps[:, sl], Wk[:, hc, ts(gt, P)], hpT[:, hc, sl],
                                start=(hc == 0), stop=(hc == HC - 1))
                    nc.scalar.copy(UkT[:, k * 2 + gt, 0:n2], ps[:, 0:n2])
                    nc.vector.tensor_copy(UkT[:, k * 2 + gt, n2:N],
                                          ps[:, n2:N])
            # root MLP hidden: Gg = gelu(W_r1 h^T + b_r1)
            for gt in range(HC):
                ps = paA.tile([P, N], dt, tag="pbig")
                for ch in range(2):
                    sl = slice(ch * n2, (ch + 1) * n2)
                    for hc in range(HC):
                        nc.tensor.matmul(
                            ps[:, sl], Wr1Ts[:, hc, ts(gt, P)], hpT[:, hc, sl],
                            start=(hc == 0), stop=(hc == HC - 1))
                nc.scalar.activation(Gg[:, gt, :], ps, GELU, bias=br1s[:, gt, :])

        with (
            tc.tile_pool(name="paS", bufs=1, space=MemorySpace.PSUM) as paS,
            tc.tile_pool(name="ptr0", bufs=2, space=MemorySpace.PSUM) as ptr0,
        ):
            # head/dep pair tiles [2,N]: J_k=[dep_k;1], I_k=[1;head_k+b_k]
            for g in range(4):
                psb = paS.tile([2, N], dt, tag="pbc")
                for ch in range(2):
                    sl = slice(ch * n2, (ch + 1) * n2)
                    for hc in range(HC):
                        nc.tensor.matmul(psb[:, sl],
                                         Whds[:, hc, 2 * g:2 * g + 2],
                                         hpT[:, hc, sl],
                                         start=(hc == 0), stop=(hc == HC - 1))
                nc.vector.tensor_scalar_add(bcg[:, g, :], psb, bcb[:, g:g + 1])
            # root scores row + exp + transposed columns
            psr = paS.tile([2, N], dt, tag="psr")
            for ch in range(2):
                sl = slice(ch * n2, (ch + 1) * n2)
                for gt in range(HC):
                    nc.tensor.matmul(psr[0:1, sl], Wr2Ts[:, gt, :],
                                     Gg[:, gt, sl],
                                     start=(gt == 0), stop=(gt == HC - 1))
            nc.scalar.activation(rs_sb, psr[0:1, :], AF.Identity,
                                 bias=bvecs[:, 2:3])
            nc.scalar.activation(exp_rs, rs_sb, AF.Exp)
            nc.vector.scalar_tensor_tensor(
                out=scrN, in0=onehot, scalar=1.0, in1=rs_sb,
                op0=OP.mult, op1=OP.mult, accum_out=gold_root)
            # gold head/dep/bias dots vs bc pair rows
            for g in range(4):
                nc.vector.scalar_tensor_tensor(
                    out=scr2, in0=cms[:, g, :], scalar=1.0, in1=bcg[:, g, :],
                    op0=OP.mult, op1=OP.mult, accum_out=gdots[:, g:g + 1])
            for it in range(NB):
                tp = ptr0.tile([P, 1], dt, tag="tp")
                nc.tensor.transpose(tp, exp_rs[:, ts(it, P)], one11)
                # negate: Mrow' root column is -exp(rs)
                nc.vector.tensor_scalar_mul(erc[:, it:it + 1], tp, -1.0)

        # ========== main loop: compat tiles + low-latency blocked LU ======= #
        with (
            tc.tile_pool(name="pck", bufs=1, space=MemorySpace.PSUM) as pck,
            tc.tile_pool(name="ppa", bufs=1, space=MemorySpace.PSUM) as ppa,
            tc.tile_pool(name="pdm", bufs=1, space=MemorySpace.PSUM) as pdm,
            tc.tile_pool(name="paw", bufs=1, space=MemorySpace.PSUM) as paw,
            tc.tile_pool(name="pws", bufs=1, space=MemorySpace.PSUM) as pws,
            tc.tile_pool(name="ptr", bufs=1, space=MemorySpace.PSUM) as ptr,
            tc.tile_pool(name="ee", bufs=2) as eep,
            tc.tile_pool(name="fp", bufs=2) as fp,
            tc.tile_pool(name="sp", bufs=3) as sp,
        ):
            # dummy-warmer bank; tail slices host the tiny grounding
            # matmuls (column sums + broadcast)
            dmt = pdm.tile([P, n2], dt, tag="dm")
            for it in range(NB):
                last = it == NB - 1
                mj = P - 1 if last else P
                itsl = ts(it, P)
                m_it = N - it * P          # diag + wide width
                # ---------- compat^T tile it: [128 j, 1024 i] --------------- #
                ck = pck.tile([P, N], dt, tag="ck")
                es1 = eep.tile([P, N], bf, tag="es1")
                r0 = sp.tile([P, 1], dt, tag="r0")
                r1 = sp.tile([P, 1], dt, tag="r1")
                for k in range(2):
                    for ch in range(2):
                        sl = slice(ch * n2, (ch + 1) * n2)
                        for gt in range(HC):
                            nc.tensor.matmul(
                                ck[:, sl], hpT[:, gt, itsl],
                                UkT[:, k * 2 + gt, sl],
                                start=(gt == 0), stop=False)
                        nc.tensor.matmul(
                            ck[:, sl], bcg[:, 2 * k, itsl],
                            bcg[:, 2 * k + 1, sl],
                            start=False, stop=True)
                    with tc.high_priority():
                        if k == 0:
                            nc.scalar.activation(Mrow[:, it, :], ck, AF.Exp,
                                                 accum_out=r0)
                        else:
                            nc.scalar.activation(es1, ck, AF.Exp,
                                                 accum_out=r1)
                # Mrow' = e0 + e1 (bf16 2x TT, in place)
                nc.vector.tensor_add(Mrow[:, it, :], Mrow[:, it, :], es1)
                nc.vector.scalar_tensor_tensor(
                    out=cs_neg[:, it:it + 1], in0=r0, scalar=-1.0, in1=r1,
                    op0=OP.mult, op1=OP.subtract)
                blk = Mrow[:, it, itsl]
                nc.vector.scalar_tensor_tensor(
                    out=blk, in0=eye_bf, scalar=cs_neg[:, it:it + 1], in1=blk,
                    op0=OP.mult, op1=OP.add)
                nc.vector.tensor_copy(Mrow[:, it, N - 1:N], erc[:, it:it + 1])
                # ---------- grounding constant c (from pre-Schur diag) ------ #
                if last:
                    nc.any.memset(rdg, 0.0)
                nc.vector.tensor_reduce(
                    rdg[:mj, 0:1], Mrow[:mj, it, it * P:it * P + mj],
                    AX.X, OP.add)
                nc.vector.scalar_tensor_tensor(
                    out=scrB[:mj, :mj], in0=eye_bf[:mj, :mj], scalar=1.0,
                    in1=blk[:mj, :mj],
                    op0=OP.mult, op1=OP.mult, accum_out=rdg[:mj, 1:2])
                nc.gpsimd.partition_all_reduce(rdsum, rdg, channels=P,
                                               reduce_op=RED.add)
                # c = (sum_diag' - sum_all') * -fac  (primed signs fold in)
                fac = (NB / (NB - it)) / (mj * (mj - 1))
                tcg = sp.tile([P, 1], dt, tag="tcg")
                nc.vector.tensor_sub(tcg, rdsum[:, 1:2], rdsum[:, 0:1])
                nc.vector.tensor_scalar_mul(c_cols[:, it:it + 1], tcg, -fac)
                if not last:
                    # Gbase = c*1 - blk' (pre-merge part of grounded diag)
                    nc.vector.scalar_tensor_tensor(
                        out=Gbase[:, it, :], in0=ones128,
                        scalar=c_cols[:, it:it + 1], in1=blk,
                        op0=OP.mult, op1=OP.subtract)
                # ---------- gold bilinear PE filler (needs hN/Q DMAs) ------- #
                if 2 <= it <= 5:
                    g = it - 2
                    kq, ht = g // 2, g % 2
                    Qs = Q0s if kq == 0 else Q1s
                    Ws = W0s if kq == 0 else W1s
                    stile = pws.tile([P, n2], dt, tag="ws")
                    spp = stile[:, 0:H]
                    for jt in range(NB):
                        nc.tensor.matmul(spp, Qs[:, jt, ts(ht, P)],
                                         hNs[:, jt, :],
                                         start=(jt == 0), stop=(jt == NB - 1))
                    nc.vector.scalar_tensor_tensor(
                        out=scrS, in0=Ws[:, ht, :], scalar=1.0, in1=spp,
                        op0=OP.mult, op1=OP.mult,
                        accum_out=sacc[:, g:g + 1])

                # ---------- At transposes (base C blocks, off-path) --------- #
                Att = None
                if it >= 1:
                    Att = fp.tile([P, 7, P], bf, tag="At")
                    for b in range(it):
                        tps = ptr.tile([P, P], bf, tag="tr")
                        nc.tensor.transpose(tps, Mrow[:, it, ts(b, P)], eye_bf)
                        if b % 2 == 0:
                            nc.scalar.copy(Att[:, b, :], tps)
                        else:
                            nc.vector.tensor_copy(Att[:, b, :], tps)

                # ---------- merge chain (transposed-C reconstruction) ------- #
                # Ct_b persists in SBUF, so slot b's accumulation
                # sum_kb<b (W_kb[:,b])^T Ct_kb runs as ONE consecutive PSUM
                # group right before its consumption: group opens and closes
                # within a step (PSUM groups can't be read mid-accumulation),
                # alternating between two banks.  Only the group's LAST
                # matmul is on the serial chain.
                accW = None
                if it >= 1:
                    accW = paw.tile([P, n2], dt, tag="aw")
                    pacc = None
                    if it >= 2:
                        pacc = ppa.tile([P, N], dt, tag="pacc")
                    Ctt = fp.tile([P, 7, P], bf, tag="Ct")
                    # first pass covers diag+wide up to one bank; the tail
                    # (it<=3) runs as a second pass through the same slot
                    wch = [(0, min(m_it, n2))]
                    for b in range(it):
                        if b == 0:
                            Ct = Att[:, 0, :]
                        else:
                            # slot group for this step: all kb<b terms;
                            # slots ping-pong between the two banks so a
                            # group can open with old terms while the
                            # previous slot (other bank) is being read.
                            # Only the LAST matmul is on the serial chain.
                            slot = pacc[:, (b % 2) * n2:(b % 2) * n2 + P]
                            for kb in range(b):
                                nc.tensor.matmul(
                                    slot, Wps[:, kb, ts(b, P)],
                                    (Att[:, 0, :] if kb == 0
                                     else Ctt[:, kb, :]),
                                    start=(kb == 0), stop=(kb == b - 1))
                            Ct = Ctt[:, b, :]
                            with tc.high_priority():
                                nc.vector.scalar_tensor_tensor(
                                    out=Ct, in0=slot, scalar=1.0,
                                    in1=Att[:, b, :],
                                    op0=OP.mult, op1=OP.add)
                        # diag+wide accumulation (one matmul per bank per kb)
                        for (o, wd) in wch:
                            nc.tensor.matmul(
                                accW[:, o:o + wd], Ct,
                                Wps[:, b, it * P + o:it * P + o + wd],
                                start=(b == 0), stop=(b == it - 1))

                # ---------- elimination of block it ------------------------- #
                if last:
                    # merged diag block, bordered handling
                    blk7 = fp.tile([P, P], bf, tag="blk7")
                    nc.vector.scalar_tensor_tensor(
                        out=blk7, in0=accW[:, 0:P], scalar=1.0, in1=blk,
                        op0=OP.mult, op1=OP.add)
                    smt = pws.tile([P, n2], dt, tag="ws")
                    rtp = smt[:, 0:1]
                    nc.tensor.matmul(rtp, blk7, eye_bf[:, P - 1:P],
                                     start=True, stop=True)
                    rvec = sp.tile([P, 1], dt, tag="rvec")
                    nc.vector.tensor_scalar_mul(rvec, rtp, -1.0)
                    cvec_sb = sp.tile([P, 1], dt, tag="cvec")
                    nc.vector.tensor_scalar_mul(cvec_sb, blk7[:, P - 1:P], -1.0)
                    nc.vector.tensor_copy(blk7[:, P - 1:P], w127n_col)
                    G = fp.tile([P, P], bf, tag="G")
                    nc.vector.scalar_tensor_tensor(
                        out=G, in0=uu, scalar=c_cols[:, it:it + 1], in1=blk7,
                        op0=OP.mult, op1=OP.subtract)
                elif it == 0:
                    G = Gbase[:, 0, :]
                else:
                    G = fp.tile([P, P], bf, tag="G")
                    with tc.high_priority():
                        nc.vector.scalar_tensor_tensor(
                            out=G, in0=accW[:, 0:P], scalar=-1.0,
                            in1=Gbase[:, it, :], op0=OP.mult, op1=OP.add)
                with tc.high_priority():
                    d = sp.tile([P, 1], dt, tag="d")
                    nc.vector.scalar_tensor_tensor(
                        out=scrB, in0=eye_bf, scalar=1.0, in1=G,
                        op0=OP.mult, op1=OP.mult, accum_out=d)
                    rinv = sp.tile([P, 1], dt, tag="rinv")
                    nc.vector.reciprocal(rinv, d)
                    F = fp.tile([P, P], bf, tag="F")
                    nc.vector.scalar_tensor_tensor(
                        out=F, in0=G, scalar=rinv, in1=eye_bf,
                        op0=OP.mult, op1=OP.subtract)
                    tps = ptr.tile([P, P], bf, tag="tr")
                    nc.tensor.transpose(tps, F, eye_bf)
                lnd = sp.tile([P, 1], dt, tag="lnd")
                nc.scalar.activation(lnd, d, AF.Ln)
                nc.vector.tensor_add(ld_acc, ld_acc, lnd)
                # merged off-diag row MB' = base + acc (fills the transpose
                # wait on DVE); wide tail (it<=3) via a second one-bank pass
                MBv = None
                if not last:
                    m = m_it - P
                    if it == 0:
                        MBv = Mrow[:, 0, P:N]
                    else:
                        MB = eep.tile([P, 768], bf, tag="MB")
                        mA = min(m_it, n2) - P
                        nc.vector.scalar_tensor_tensor(
                            out=MB[:, 0:mA], in0=accW[:, P:P + mA],
                            scalar=1.0,
                            in1=Mrow[:, it, (it + 1) * P:(it + 1) * P + mA],
                            op0=OP.mult, op1=OP.add)
                        if m_it > n2:
                            mB = m_it - n2
                            accB = paw.tile([P, n2], dt, tag="aw")
                            for b2 in range(it):
                                nc.tensor.matmul(
                                    accB[:, 0:mB],
                                    (Att[:, 0, :] if b2 == 0
                                     else Ctt[:, b2, :]),
                                    Wps[:, b2, it * P + n2:N],
                                    start=(b2 == 0), stop=(b2 == it - 1))
                            nc.vector.scalar_tensor_tensor(
                                out=MB[:, mA:m], in0=accB[:, 0:mB],
                                scalar=1.0,
                                in1=Mrow[:, it, it * P + n2:N],
                                op0=OP.mult, op1=OP.add)
                        MBv = MB[:, 0:m]
                Ft = fp.tile([P, P], bf, tag="Ft")
                with tc.high_priority():
                    nc.scalar.copy(Ft, tps)
                t2 = sp.tile([P, 1], dt, tag="t2")
                nc.vector.scalar_tensor_tensor(
                    out=scrB, in0=F, scalar=-0.5, in1=Ft,
                    op0=OP.mult, op1=OP.mult, accum_out=t2)
                nc.vector.tensor_add(ld_acc, ld_acc, t2)

                if not last:
                    # critical W path: order-1 Neumann WITHOUT the rank-1
                    # det-lemma W-correction (logdet keeps its correction
                    # below; the W impact is higher-order and within the
                    # error budget) -> chain is G,d,rinv,F,Ft,PnTFs,W only.
                    with tc.high_priority():
                        PnTF = fp.tile([P, P], bf, tag="PnTF")
                        nc.vector.scalar_tensor_tensor(
                            out=PnTF, in0=Ft, scalar=-1.0, in1=eye_bf,
                            op0=OP.mult, op1=OP.add)
                        PnTFs = fp.tile([P, P], bf, tag="PnTFs")
                        nc.vector.tensor_scalar_mul(PnTFs, PnTF, rinv)
                        m = m_it - P
                        wps = pws.tile([P, n2], dt, tag="ws")
                        nc.tensor.matmul(wps[:, 0:P], PnTFs, MBv[:, 0:P],
                                         start=True, stop=True)
                        nc.scalar.copy(Wps[:, it, (it + 1) * P:
                                           (it + 2) * P], wps[:, 0:P])
                    for (o, wd) in _wchunks(m)[1:]:
                        wps = pws.tile([P, n2], dt, tag="ws")
                        nc.tensor.matmul(wps[:, :wd], PnTFs, MBv[:, o:o + wd],
                                         start=True, stop=True)
                        nc.scalar.copy(Wps[:, it, (it + 1) * P + o:
                                           (it + 1) * P + o + wd],
                                       wps[:, :wd])
                    # det-lemma logdet scalar correction dropped for
                    # interior blocks: the grounding shift contributes
                    # ~1e-4 relative, far under the 2e-2 budget
                else:
                    # bordered last block, root in column 127
                    smt2 = pws.tile([P, n2], dt, tag="ws")
                    F2p = smt2[:, 0:P]
                    nc.tensor.matmul(F2p, F, Ft, start=True, stop=True)
                    PnT2 = fp.tile([P, P], bf, tag="PnT2")
                    nc.vector.scalar_tensor_tensor(
                        out=PnT2, in0=Ft, scalar=-1.0, in1=eye_bf,
                        op0=OP.mult, op1=OP.add)
                    nc.vector.tensor_add(PnT2, PnT2, F2p)
                    x0 = sp.tile([P, 1], dt, tag="x0")
                    nc.vector.tensor_mul(x0, rinv, u_col)
                    x0_bf = sp.tile([P, 1], bf, tag="x0b")
                    nc.vector.tensor_copy(x0_bf, x0)
                    qp = smt2[:, P:P + 1]
                    nc.tensor.matmul(qp, PnT2, x0_bf, start=True, stop=True)
                    qm = sp.tile([P, 1], dt, tag="qm")
                    nc.vector.tensor_mul(qm, qp, u_col)
                    chat = sp.tile([P, 1], dt, tag="chat")
                    nc.vector.tensor_mul(chat, cvec_sb, u_col)
                    x0c = sp.tile([P, 1], dt, tag="x0c")
                    nc.vector.tensor_mul(x0c, rinv, chat)
                    x0c_bf = sp.tile([P, 1], bf, tag="x0cb")
                    nc.vector.tensor_copy(x0c_bf, x0c)
                    y1p = smt2[:, P + 1:P + 2]
                    nc.tensor.matmul(y1p, PnT2, x0c_bf, start=True, stop=True)
                    y1m = sp.tile([P, 1], dt, tag="y1m")
                    nc.vector.tensor_mul(y1m, y1p, u_col)
                    dots = smt2[0:1, 2 * P:2 * P + 5]
                    nc.tensor.matmul(dots[:, 0:1], y1m, ones_col,
                                     start=True, stop=True)
                    nc.tensor.matmul(dots[:, 1:2], rvec, y1m,
                                     start=True, stop=True)
                    nc.tensor.matmul(dots[:, 2:3], rvec, qm,
                                     start=True, stop=True)
                    nc.tensor.matmul(dots[:, 3:4], qm, ones_col,
                                     start=True, stop=True)
                    nc.tensor.matmul(dots[:, 4:5], rvec, eyef[:, P - 1:P],
                                     start=True, stop=True)
                    dsb = sp.tile([1, 5], dt, tag="dsb")
                    nc.vector.tensor_copy(dsb, dots)
                    tac = sp.tile([1, 1], dt, tag="tac")
                    nc.vector.tensor_mul(tac, dsb[:, 3:4],
                                         c_cols[0:1, it:it + 1])
                    detr = sp.tile([1, 1], dt, tag="detr")
                    nc.vector.tensor_scalar(
                        out=detr, in0=tac, scalar1=-1.0, scalar2=1.0,
                        op0=OP.mult, op1=OP.add)
                    lndr = sp.tile([1, 1], dt, tag="lndr")
                    nc.scalar.activation(lndr, detr, AF.Ln)
                    nc.vector.tensor_add(ld_acc[0:1, :], ld_acc[0:1, :], lndr)
                    invdr = sp.tile([1, 1], dt, tag="invdr")
                    nc.vector.reciprocal(invdr, detr)
                    gam = sp.tile([1, 1], dt, tag="gam")
                    nc.vector.tensor_mul(gam, c_cols[0:1, it:it + 1], invdr)
                    bg = sp.tile([1, 1], dt, tag="bg")
                    nc.vector.tensor_mul(bg, dsb[:, 0:1], gam)
                    t3 = sp.tile([1, 1], dt, tag="t3")
                    nc.vector.tensor_mul(t3, bg, dsb[:, 2:3])
                    t4 = sp.tile([1, 1], dt, tag="t4")
                    nc.vector.tensor_sub(t4, dsb[:, 4:5], dsb[:, 1:2])
                    sca = sp.tile([1, 1], dt, tag="sca")
                    nc.vector.tensor_sub(sca, t4, t3)
                    lnsc = sp.tile([1, 1], dt, tag="lnsc")
                    nc.scalar.activation(lnsc, sca, AF.Ln)
                    nc.vector.tensor_add(ld_acc[0:1, :], ld_acc[0:1, :], lnsc)

            # HAM warm-keeper: lowest-priority dummy matmuls that the
            # scheduler slots into PE idle gaps, keeping the activity
            # monitor from re-throttling the PE clock to 1.2 GHz.  Each
            # batch reads its iteration's Mrow tile so late batches cannot
            # be hoisted early (supply survives into the tail).
            for it, ndm in enumerate((8, 5, 6, 6, 8, 8, 9, 10)):
                for _ in range(ndm):
                    nc.tensor.matmul(dmt[:, 0:64], eye_bf, Mrow[:, it, 0:64],
                                     start=True, stop=True)

        # ================= gold bilinear + finale ========================== #
        with (
            tc.tile_pool(name="pf", bufs=1, space=MemorySpace.PSUM) as pf,
            tc.tile_pool(name="fin", bufs=1) as finp,
        ):
            sg = finp.tile([P, 1], dt, tag="sg")
            nc.vector.tensor_add(sg, sacc[:, 0:1], sacc[:, 1:2])
            nc.vector.tensor_add(sg, sg, sacc[:, 2:3])
            nc.vector.tensor_add(sg, sg, sacc[:, 3:4])
            fin = pf.tile([1, 8], dt, tag="fin")
            nc.tensor.matmul(fin[:, 0:1], ld_acc, ones_col, start=True, stop=True)
            nc.tensor.matmul(fin[:, 1:2], sg, ones_col, start=True, stop=True)
            gsum = finp.tile([2, 1], dt, tag="gsum")
            nc.vector.tensor_reduce(gsum, gdots, AX.X, OP.add)
            nc.tensor.matmul(fin[:, 2:3], gsum, ones_col[0:2, :],
                             start=True, stop=True)
            fsb = finp.tile([1, 3], dt, tag="fsb")
            nc.vector.tensor_copy(fsb, fin[:, 0:3])
            f1 = finp.tile([1, 1], dt, tag="f1")
            nc.vector.tensor_sub(f1, fsb[:, 0:1], fsb[:, 1:2])
            f2 = finp.tile([1, 1], dt, tag="f2")
            nc.vector.tensor_sub(f2, f1, fsb[:, 2:3])
            f3 = finp.tile([1, 1], dt, tag="f3")
            nc.vector.tensor_sub(f3, f2, gold_root)
            out_sb = finp.tile([1, 1], dt, tag="out")
            nc.scalar.activation(out_sb, f3, AF.Relu)
            nc.sync.dma_start(loss_d[:, :], out_sb)

    _CACHE["nc"] = nc
    return nc


def finalize_nc(nc):
    """Prepare nc for NEFF compilation (mutates the module; sim-incompatible)."""
    if getattr(nc, "_finalized", False):
        return nc
    from concourse import mybir

    mybir.codegen_inst_isa_subclasses(nc)
    fixed_json = _split_multi_waits(nc.to_json_bytes())
    nc.to_json_bytes = lambda: fixed_json
    nc._finalized = True
    return nc


# --------------------------------------------------------------------------- #
# host-side sharding / prep
# --------------------------------------------------------------------------- #
def _cvec():
    c = np.zeros((P, 3), F32)
    c[:, 0] = 1.0
    c[P - 1, 0] = 0.0
    c[P - 1, 1] = 1.0
    c[P - 1, 2] = -1.0
    return c


def prep_in_maps(inputs):
    import ml_dtypes

    BF = ml_dtypes.bfloat16
    h_cat = np.asarray(inputs["h_cat"], F32)
    left = np.asarray(inputs["left_adj"], F32)
    right = np.asarray(inputs["right_adj"], F32)
    roots = np.asarray(inputs["roots"])
    W_bilin = np.asarray(inputs["W_bilin"], F32)
    b_bilin = np.asarray(inputs["b_bilin"], F32)
    W_head = np.asarray(inputs["W_head"], F32)
    W_dep = np.asarray(inputs["W_dep"], F32)
    W_r1 = np.asarray(inputs["W_r1"], F32)
    b_r1 = np.asarray(inputs["b_r1"], F32)
    W_r2 = np.asarray(inputs["W_r2"], F32)
    b_r2 = np.asarray(inputs["b_r2"], F32)

    # Whd col pairs -> [dep0,0], [0,head0], [dep1,0], [0,head1]
    z = np.zeros(H, F32)
    whd = np.stack([W_dep[0], z, z, W_head[0], W_dep[1], z, z, W_head[1]],
                   axis=1)
    bcb = np.array([[0.0, 1.0, 0.0, 1.0],
                    [1.0, b_bilin[0], 1.0, b_bilin[1]]], F32)
    uu = np.ones((P, P), F32)
    uu[:, P - 1] = 0.0
    uu[P - 1, :] = 0.0
    shared = {
        "W0": np.ascontiguousarray(W_bilin[0]).astype(BF),
        "W1": np.ascontiguousarray(W_bilin[1]).astype(BF),
        "Whd": np.ascontiguousarray(whd).astype(BF),
        "bcb": bcb,
        "uu": uu.astype(BF),
        "Wr1T": np.ascontiguousarray(W_r1.T).astype(BF),
        "br1": np.ascontiguousarray(b_r1.reshape(H, 1)),
        "Wr2T": np.ascontiguousarray(W_r2.reshape(1, H).T).astype(BF),
        "bvec": np.ascontiguousarray(
            np.array([b_bilin[0], b_bilin[1], b_r2.reshape(-1)[0], 0.0],
                     F32).reshape(1, 4)),
        "cvec": _cvec(),
    }
    in_maps = []
    idx = np.arange(N)
    for b in range(B):
        hp = np.roll(h_cat[b], -1, axis=0)
        Lp = np.roll(np.roll(left[b], -1, axis=0), -1, axis=1)
        Rp = np.roll(np.roll(right[b], -1, axis=0), -1, axis=1)
        par = np.argmax(Lp + Rp, axis=0)
        mask0 = Lp[par, idx] > 0
        mask1 = Rp[par, idx] > 0
        Q0 = np.where(mask0[:, None], hp[par], 0).astype(F32)
        Q1 = np.where(mask1[:, None], hp[par], 0).astype(F32)
        # cm pairs: J_k -> [mask_k; 0], I_k -> [0; cnt_k]
        cm = np.zeros((2, 4, N), F32)
        cm[0, 0] = mask0.astype(F32)
        cm[1, 1] = np.bincount(par[mask0], minlength=N)
        cm[0, 2] = mask1.astype(F32)
        cm[1, 3] = np.bincount(par[mask1], minlength=N)
        cm = cm.reshape(2, 4 * N)
        onehot = np.zeros((1, N), F32)
        onehot[0, (int(roots[b]) - 1) % N] = 1.0
        m = dict(shared)
        m["hpT"] = np.ascontiguousarray(hp.T).astype(BF)
        m["hN"] = np.ascontiguousarray(hp).astype(BF)
        m["Q0"] = np.ascontiguousarray(Q0).astype(BF)
        m["Q1"] = np.ascontiguousarray(Q1).astype(BF)
        m["cm"] = np.ascontiguousarray(cm).astype(BF)
        m["onehot"] = onehot
        in_maps.append(m)
    return in_maps


def kernel(**inputs):
    global LAST_RESULTS
    nc = finalize_nc(build_nc())
    in_maps = prep_in_maps(inputs)
    from concourse.bass_utils import run_bass_kernel_spmd

    trace = bool(os.environ.get("KERNEL_TRACE"))
    res = run_bass_kernel_spmd(nc, in_maps, list(range(B)), trace=trace)
    LAST_RESULTS = res
    losses = np.array([res.results[i]["loss"][0, 0] for i in range(B)], F32)
    return np.asarray(F32(ALPHA) * losses.sum(dtype=F32) / F32(B))


# revision 29
# speedup vs baseline: 1.0196x; 1.0104x over previous
"""Trainium2 Bass kernel for nn_DependencyTreeModel (dependency-tree matrix-tree loss).

Strategy (data-parallel over batch B=8, one batch element per NeuronCore):
  * Host: permute node 0 to the end (symmetric permutation, det-invariant),
    gather parent rows Qk = h[parent] masked by side (no model FLOPs),
    ship everything bf16 where precision allows.
  * Device, per core, TRANSPOSED space (M = L^T, det-invariant), and
    SIGN-FLIPPED storage (M' = -M; det unchanged since N=1024 is even):
      - biaffine compat^T channels in PSUM via bf16 PE matmuls; exp with
        fused row-sum accumulation; Mrow' = e0+e1 via one bf16 2x TT.
      - blocked LU with grounding G = T + c*11^T (c from pre-Schur diag via
        gpsimd partition_all_reduce), order-1 Neumann + rank-1 det-lemma,
        tr2-only trace-log series.
      - Schur merges RESTRUCTURED for latency: per tile the column-block
        chain C_b is reconstructed TRANSPOSED via PSUM accumulators
        PaccT[b] = sum_kb (W_kb[:,b])^T @ Ct_kb  (stationary = stored W
        slices, so no transposes or copies on the serial chain; chain step
        is one 128-wide matmul + one 128-wide STT).  The wide updates and
        the diag update accumulate in PSUM across kb and are applied with
        a single STT per tile, feeding the W matmuls directly (rinv folded
        into the row-scaled PnTF stationary).
      - loss_b = relu(logdet - gold); host sums: ALPHA * sum(loss_b) / B.
"""
import os
import sys

sys.path.insert(0, "/opt/trn_rl_repo")

import numpy as np

B, N, H = 8, 1024, 256
P = 128
NB = N // P  # 8
HC = H // P  # 2
n2 = N // 2  # 512
ALPHA = 0.25
F32 = np.float32

_CACHE = {}
LAST_RESULTS = None


def _split_multi_waits(bir_bytes, max_waits=1):
    """walrus in this container accepts at most one sync wait per instruction;
    hoist extra waits onto preceding sequencer NoOps (same engine, in order)."""
    import orjson

    d = orjson.loads(bir_bytes)
    for func in d["functions"]:
        for blk in func["blocks"]:
            insts = blk.get("instructions")
            if not insts:
                continue
            new = []
            for ins in insts:
                si = ins.get("sync_info")
                ow = (si or {}).get("on_wait") or []
                if len(ow) > max_waits and ins.get("engine", "Unassigned") != "Unassigned":
                    head, keep = ow[:-max_waits], ow[-max_waits:]
                    for i, w in enumerate(head):
                        nop = {"engine": ins["engine"], "ins": [], "outs": [],
                               "name": f'{ins["name"]}-sw{i}', "opcode": "NoOp",
                               "sync_info": {"on_wait": [w], "on_update": []}}
                        if "debug" in ins:
                            nop["debug"] = ins["debug"]
                        new.append(nop)
                    si["on_wait"] = keep
                new.append(ins)
            blk["instructions"] = new
    return orjson.dumps(d)


def _wchunks(m):
    """W-matmul column chunks: peel 128 head (so the next tile's last merge
    matmuls can start early), then <=512 pieces."""
    out = []
    if m > 128:
        out.append((0, 128))
        o = 128
    else:
        return [(0, m)]
    while o < m:
        w = min(512, m - o)
        out.append((o, w))
        o += w
    return out


# --------------------------------------------------------------------------- #
# device program
# --------------------------------------------------------------------------- #
def build_nc():
    if "nc" in _CACHE:
        return _CACHE["nc"]

    import concourse.bass as bass
    from concourse import bass_isa, library_config
    import concourse.mybir as mybir
    from concourse.bass import MemorySpace, ts
    from concourse.masks import make_identity
    from concourse.tile import TileContext

    dt = mybir.dt.float32
    bf = mybir.dt.bfloat16
    AF = mybir.ActivationFunctionType
    OP = mybir.AluOpType
    AX = mybir.AxisListType
    RED = bass_isa.ReduceOp
    # CoreSim lacks Gelu; swap for Identity under KERNEL_SIM_NOGELU (the
    # numpy sim reference applies the same substitution)
    GELU = AF.Identity if os.environ.get("KERNEL_SIM_NOGELU") else AF.Gelu

    nc = bass.Bass()

    hpT_d = nc.declare_dram_parameter("hpT", [H, N], bf, isOutput=False)
    hN_d = nc.declare_dram_parameter("hN", [N, H], bf, isOutput=False)
    Q0_d = nc.declare_dram_parameter("Q0", [N, H], bf, isOutput=False)
    Q1_d = nc.declare_dram_parameter("Q1", [N, H], bf, isOutput=False)
    W0_d = nc.declare_dram_parameter("W0", [H, H], bf, isOutput=False)
    W1_d = nc.declare_dram_parameter("W1", [H, H], bf, isOutput=False)
    Whd_d = nc.declare_dram_parameter("Whd", [H, 8], bf, isOutput=False)
    bcb_d = nc.declare_dram_parameter("bcb", [2, 4], dt, isOutput=False)
    uu_d = nc.declare_dram_parameter("uu", [P, P], bf, isOutput=False)
    Wr1T_d = nc.declare_dram_parameter("Wr1T", [H, H], bf, isOutput=False)
    br1_d = nc.declare_dram_parameter("br1", [H, 1], dt, isOutput=False)
    Wr2T_d = nc.declare_dram_parameter("Wr2T", [H, 1], bf, isOutput=False)
    bvec_d = nc.declare_dram_parameter("bvec", [1, 4], dt, isOutput=False)
    cm_d = nc.declare_dram_parameter("cm", [2, 4 * N], bf, isOutput=False)
    onehot_d = nc.declare_dram_parameter("onehot", [1, N], dt, isOutput=False)
    cvec_d = nc.declare_dram_parameter("cvec", [P, 3], dt, isOutput=False)
    loss_d = nc.declare_dram_parameter("loss", [1, 1], dt, isOutput=True)

    from contextlib import ExitStack

    with TileContext(nc) as tc, ExitStack() as stack:
        consts = stack.enter_context(tc.tile_pool(name="consts", bufs=1))

        # ---- persistent SBUF tensors; critical-path DMAs first (sync q),
        # late-use bulk (hN/Q0/Q1) on the scalar hwdge queue ----
        hpT = consts.tile([P, HC, N], bf)
        nc.sync.dma_start(hpT, hpT_d.rearrange("(hc p) n -> p hc n", p=P))
        W0s = consts.tile([P, HC, H], bf)
        nc.scalar.dma_start(W0s, W0_d.rearrange("(hc p) g -> p hc g", p=P))
        W1s = consts.tile([P, HC, H], bf)
        nc.scalar.dma_start(W1s, W1_d.rearrange("(hc p) g -> p hc g", p=P))
        Wr1Ts = consts.tile([P, HC, H], bf)
        nc.scalar.dma_start(Wr1Ts, Wr1T_d.rearrange("(hc p) g -> p hc g", p=P))
        Whds = consts.tile([P, HC, 8], bf)
        nc.scalar.dma_start(Whds, Whd_d.rearrange("(hc p) m -> p hc m", p=P))
        bcb = consts.tile([2, 4], dt)
        nc.sync.dma_start(bcb, bcb_d[:, :])
        br1s = consts.tile([P, HC, 1], dt)
        nc.sync.dma_start(br1s, br1_d.rearrange("(hc p) o -> p hc o", p=P))
        Wr2Ts = consts.tile([P, HC, 1], bf)
        nc.scalar.dma_start(Wr2Ts, Wr2T_d.rearrange("(hc p) o -> p hc o", p=P))
        bvecs = consts.tile([1, 4], dt)
        nc.sync.dma_start(bvecs, bvec_d[:, :])
        cvecs = consts.tile([P, 3], dt)
        nc.sync.dma_start(cvecs, cvec_d[:, :])
        uu = consts.tile([P, P], bf)   # ones with row 127 & col 127 zeroed
        nc.sync.dma_start(uu, uu_d[:, :])
        onehot = consts.tile([1, N], dt)
        nc.sync.dma_start(onehot, onehot_d[:, :])
        # bulk, needed late (gold bilinear at it>=2)
        Q0s = consts.tile([P, NB, H], bf)
        nc.sync.dma_start(Q0s, Q0_d.rearrange("(jt p) h -> p jt h", p=P))
        cms = consts.tile([2, 4, N], bf)
        nc.sync.dma_start(cms, cm_d.rearrange("p (g n) -> p g n", g=4))
        hNs = consts.tile([P, NB, H], bf)
        nc.scalar.dma_start(hNs, hN_d.rearrange("(jt p) h -> p jt h", p=P))
        Q1s = consts.tile([P, NB, H], bf)
        nc.scalar.dma_start(Q1s, Q1_d.rearrange("(jt p) h -> p jt h", p=P))

        u_col = cvecs[:, 0:1]          # ones, 0 at 127
        w127n_col = cvecs[:, 2:3]      # -e127

        eye_bf = consts.tile([P, P], bf)
        make_identity(nc, eye_bf)
        eyef = consts.tile([P, P], dt)
        make_identity(nc, eyef)
        # gpsimd ucode with partition_all_reduce (grounding constant c)
        nc.gpsimd.load_library(library_config.attn)
        ones128 = consts.tile([P, P], bf)
        nc.any.memset(ones128, 1.0)
        ones_col = consts.tile([P, 1], dt)
        nc.any.memset(ones_col, 1.0)
        ones_col_bf = consts.tile([P, 1], bf)
        nc.any.memset(ones_col_bf, 1.0)
        ones_row = consts.tile([1, P], dt)
        nc.any.memset(ones_row, 1.0)
        ones_row_bf = consts.tile([1, P], bf)
        nc.any.memset(ones_row_bf, 1.0)
        one11 = consts.tile([1, 1], dt)
        nc.any.memset(one11, 1.0)
        scratch11 = consts.tile([1, 1], dt)
        # warm the gelu table set while input DMAs stream
        nc.scalar.activation(scratch11, one11, GELU)

        UkT = consts.tile([P, 4, N], bf)     # (h W_k)^T, idx = k*2+gt
        Gg = consts.tile([P, HC, N], bf)     # gelu(h W_r1^T + b_r1) transposed
        bcg = consts.tile([2, 4, N], bf)     # pairs: J0 I0 J1 I1
        Mrow = consts.tile([P, NB, N], bf)   # M' = -(L^T): rows = +(e0+e1)
        Wps = consts.tile([P, NB, N], bf)    # per-block W'' = T^-1 Mrow'
        Gbase = consts.tile([P, NB, P], bf)  # c*1 - blk' (pre-merge, per tile)
        rs_sb = consts.tile([1, N], dt)
        exp_rs = consts.tile([1, N], dt)
        erc = consts.tile([P, NB], dt)       # -exp_rs as columns per tile
        cs_neg = consts.tile([P, NB], dt)    # -colsum per tile
        c_cols = consts.tile([P, NB], dt)    # grounding c (orig sign), bcast
        ld_acc = consts.tile([P, 1], dt)
        nc.any.memset(ld_acc, 0.0)
        gold_root = consts.tile([1, 1], dt)
        gdots = consts.tile([2, 4], dt)
        scr2 = consts.tile([2, N], bf)
        sacc = consts.tile([P, 4], dt)       # S_k reduce partials
        scrB = consts.tile([P, P], bf)       # scratch for fused reduces
        scrS = consts.tile([P, H], bf)
        scrN = consts.tile([1, N], dt)
        rdg = consts.tile([P, 2], dt)        # grounding partials
        rdsum = consts.tile([P, 2], dt)      # after partition all-reduce

        # ================= phase A: weight transforms ====================== #
        with tc.tile_pool(name="paA", bufs=2, space=MemorySpace.PSUM) as paA:
            # U_kT[g, i] = sum_h W_k[h, g] hpT[h, i]
            for k, Wk in ((0, W0s), (1, W1s)):
                for gt in range(HC):
                    ps = paA.tile([P, N], dt, tag="pbig")
                    for ch in range(2):
                        sl = slice(ch * n2, (ch + 1) * n2)
                        for hc in range(HC):
                            nc.tensor.matmul(
                                ps[:, sl], Wk[:, hc, ts(gt, P)], hpT[:, hc, sl],
                                start=(hc == 0), stop=(hc == HC - 1))
                    nc.scalar.copy(UkT[:, k * 2 + gt, 0:n2], ps[:, 0:n2])
                    nc.vector.tensor_copy(UkT[:, k * 2 + gt, n2:N],
                                          ps[:, n2:N])
            # root MLP hidden: Gg = gelu(W_r1 h^T + b_r1)
            for gt in range(HC):
                ps = paA.tile([P, N], dt, tag="pbig")
                for ch in range(2):
                    sl = slice(ch * n2, (ch + 1) * n2)
                    for hc in range(HC):
                        nc.tensor.matmul(
                            ps[:, sl], Wr1Ts[:, hc, ts(gt, P)], hpT[:, hc, sl],
                            start=(hc == 0), stop=(hc == HC - 1))
                nc.scalar.activation(Gg[:, gt, :], ps, GELU, bias=br1s[:, gt, :])

        with (
            tc.tile_pool(name="paS", bufs=1, space=MemorySpace.PSUM) as paS,
            tc.tile_pool(name="ptr0", bufs=2, space=MemorySpace.PSUM) as ptr0,
        ):
            # head/dep pair tiles [2,N]: J_k=[dep_k;1], I_k=[1;head_k+b_k]
            for g in range(4):
                psb = paS.tile([2, N], dt, tag="pbc")
                for ch in range(2):
                    sl = slice(ch * n2, (ch + 1) * n2)
                    for hc in range(HC):
                        nc.tensor.matmul(psb[:, sl],
                                         Whds[:, hc, 2 * g:2 * g + 2],
                                         hpT[:, hc, sl],
                                         start=(hc == 0), stop=(hc == HC - 1))
                nc.vector.tensor_scalar_add(bcg[:, g, :], psb, bcb[:, g:g + 1])
            # root scores row + exp + transposed columns
            psr = paS.tile([2, N], dt, tag="psr")
            for ch in range(2):
                sl = slice(ch * n2, (ch + 1) * n2)
                for gt in range(HC):
                    nc.tensor.matmul(psr[0:1, sl], Wr2Ts[:, gt, :],
                                     Gg[:, gt, sl],
                                     start=(gt == 0), stop=(gt == HC - 1))
            nc.scalar.activation(rs_sb, psr[0:1, :], AF.Identity,
                                 bias=bvecs[:, 2:3])
            nc.scalar.activation(exp_rs, rs_sb, AF.Exp)
            nc.vector.scalar_tensor_tensor(
                out=scrN, in0=onehot, scalar=1.0, in1=rs_sb,
                op0=OP.mult, op1=OP.mult, accum_out=gold_root)
            # gold head/dep/bias dots vs bc pair rows
            for g in range(4):
                nc.vector.scalar_tensor_tensor(
                    out=scr2, in0=cms[:, g, :], scalar=1.0, in1=bcg[:, g, :],
                    op0=OP.mult, op1=OP.mult, accum_out=gdots[:, g:g + 1])
            for it in range(NB):
                tp = ptr0.tile([P, 1], dt, tag="tp")
                nc.tensor.transpose(tp, exp_rs[:, ts(it, P)], one11)
                # negate: Mrow' root column is -exp(rs)
                nc.vector.tensor_scalar_mul(erc[:, it:it + 1], tp, -1.0)

        # ========== main loop: compat tiles + low-latency blocked LU ======= #
        with (
            tc.tile_pool(name="pck", bufs=1, space=MemorySpace.PSUM) as pck,
            tc.tile_pool(name="ppa", bufs=1, space=MemorySpace.PSUM) as ppa,
            tc.tile_pool(name="pdm", bufs=1, space=MemorySpace.PSUM) as pdm,
            tc.tile_pool(name="paw", bufs=1, space=MemorySpace.PSUM) as paw,
            tc.tile_pool(name="pws", bufs=1, space=MemorySpace.PSUM) as pws,
            tc.tile_pool(name="ptr", bufs=1, space=MemorySpace.PSUM) as ptr,
            tc.tile_pool(name="ee", bufs=2) as eep,
            tc.tile_pool(name="fp", bufs=2) as fp,
            tc.tile_pool(name="sp", bufs=3) as sp,
        ):
            # dummy-warmer bank; tail slices host the tiny grounding
            # matmuls (column sums + broadcast)
            dmt = pdm.tile([P, n2], dt, tag="dm")
            for it in range(NB):
                last = it == NB - 1
                mj = P - 1 if last else P
                itsl = ts(it, P)
                m_it = N - it * P          # diag + wide width
                # ---------- compat^T tile it: [128 j, 1024 i] --------------- #
                ck = pck.tile([P, N], dt, tag="ck")
                es1 = eep.tile([P, N], bf, tag="es1")
                r0 = sp.tile([P, 1], dt, tag="r0")
                r1 = sp.tile([P, 1], dt, tag="r1")
                for k in range(2):
                    for ch in range(2):
                        sl = slice(ch * n2, (ch + 1) * n2)
                        for gt in range(HC):
                            nc.tensor.matmul(
                                ck[:, sl], hpT[:, gt, itsl],
                                UkT[:, k * 2 + gt, sl],
                                start=(gt == 0), stop=False)
                        nc.tensor.matmul(
                            ck[:, sl], bcg[:, 2 * k, itsl],
                            bcg[:, 2 * k + 1, sl],
                            start=False, stop=True)
                    with tc.high_priority():
                        if k == 0:
                            nc.scalar.activation(Mrow[:, it, :], ck, AF.Exp,
                                                 accum_out=r0)
                        else:
                            nc.scalar.activation(es1, ck, AF.Exp,
                                                 accum_out=r1)
                # Mrow' = e0 + e1 (bf16 2x TT, in place)
                nc.vector.tensor_add(Mrow[:, it, :], Mrow[:, it, :], es1)
                nc.vector.scalar_tensor_tensor(
                    out=cs_neg[:, it:it + 1], in0=r0, scalar=-1.0, in1=r1,
                    op0=OP.mult, op1=OP.subtract)
                blk = Mrow[:, it, itsl]
                nc.vector.scalar_tensor_tensor(
                    out=blk, in0=eye_bf, scalar=cs_neg[:, it:it + 1], in1=blk,
                    op0=OP.mult, op1=OP.add)
                nc.vector.tensor_copy(Mrow[:, it, N - 1:N], erc[:, it:it + 1])
                # ---------- grounding constant c (from pre-Schur diag) ------ #
                if last:
                    nc.any.memset(rdg, 0.0)
                nc.vector.tensor_reduce(
                    rdg[:mj, 0:1], Mrow[:mj, it, it * P:it * P + mj],
                    AX.X, OP.add)
                nc.vector.scalar_tensor_tensor(
                    out=scrB[:mj, :mj], in0=eye_bf[:mj, :mj], scalar=1.0,
                    in1=blk[:mj, :mj],
                    op0=OP.mult, op1=OP.mult, accum_out=rdg[:mj, 1:2])
                nc.gpsimd.partition_all_reduce(rdsum, rdg, channels=P,
                                               reduce_op=RED.add)
                # c = (sum_diag' - sum_all') * -fac  (primed signs fold in)
                fac = (NB / (NB - it)) / (mj * (mj - 1))
                tcg = sp.tile([P, 1], dt, tag="tcg")
                nc.vector.tensor_sub(tcg, rdsum[:, 1:2], rdsum[:, 0:1])
                nc.vector.tensor_scalar_mul(c_cols[:, it:it + 1], tcg, -fac)
                if not last:
                    # Gbase = c*1 - blk' (pre-merge part of grounded diag)
                    nc.vector.scalar_tensor_tensor(
                        out=Gbase[:, it, :], in0=ones128,
                        scalar=c_cols[:, it:it + 1], in1=blk,
                        op0=OP.mult, op1=OP.subtract)
                # ---------- gold bilinear PE filler (needs hN/Q DMAs) ------- #
                if 2 <= it <= 5:
                    g = it - 2
                    kq, ht = g // 2, g % 2
                    Qs = Q0s if kq == 0 else Q1s
                    Ws = W0s if kq == 0 else W1s
                    stile = pws.tile([P, n2], dt, tag="ws")
                    spp = stile[:, 0:H]
                    for jt in range(NB):
                        nc.tensor.matmul(spp, Qs[:, jt, ts(ht, P)],
                                         hNs[:, jt, :],
                                         start=(jt == 0), stop=(jt == NB - 1))
                    nc.vector.scalar_tensor_tensor(
                        out=scrS, in0=Ws[:, ht, :], scalar=1.0, in1=spp,
                        op0=OP.mult, op1=OP.mult,
                        accum_out=sacc[:, g:g + 1])

                # ---------- At transposes (base C blocks, off-path) --------- #
                Att = None
                if it >= 1:
                    Att = fp.tile([P, 7, P], bf, tag="At")
                    for b in range(it):
                        tps = ptr.tile([P, P], bf, tag="tr")
                        nc.tensor.transpose(tps, Mrow[:, it, ts(b, P)], eye_bf)
                        if b % 2 == 0:
                            nc.scalar.copy(Att[:, b, :], tps)
                        else:
                            nc.vector.tensor_copy(Att[:, b, :], tps)

                # ---------- merge chain (transposed-C reconstruction) ------- #
                # Ct_b persists in SBUF, so slot b's accumulation
                # sum_kb<b (W_kb[:,b])^T Ct_kb runs as ONE consecutive PSUM
                # group right before its consumption: group opens and closes
                # within a step (PSUM groups can't be read mid-accumulation),
                # alternating between two banks.  Only the group's LAST
                # matmul is on the serial chain.
                accW = None
                if it >= 1:
                    accW = paw.tile([P, n2], dt, tag="aw")
                    pacc = None
                    if it >= 2:
                        pacc = ppa.tile([P, N], dt, tag="pacc")
                    Ctt = fp.tile([P, 7, P], bf, tag="Ct")
                    # first pass covers diag+wide up to one bank; the tail
                    # (it<=3) runs as a second pass through the same slot
                    wch = [(0, min(m_it, n2))]
                    for b in range(it):
                        if b == 0:
                            Ct = Att[:, 0, :]
                        else:
                            # slot group for this step: all kb<b terms;
                            # slots ping-pong between the two banks so a
                            # group can open with old terms while the
                            # previous slot (other bank) is being read.
                            # Only the LAST matmul is on the serial chain.
                            slot = pacc[:, (b % 2) * n2:(b % 2) * n2 + P]
                            for kb in range(b):
                                nc.tensor.matmul(
                                    slot, Wps[:, kb, ts(b, P)],
                                    (Att[:, 0, :] if kb == 0
                                     else Ctt[:, kb, :]),
                                    start=(kb == 0), stop=(kb == b - 1))
                            Ct = Ctt[:, b, :]
                            with tc.high_priority():
                                nc.vector.scalar_tensor_tensor(
                                    out=Ct, in0=slot, scalar=1.0,
                                    in1=Att[:, b, :],
                                    op0=OP.mult, op1=OP.add)
                        # diag+wide accumulation (one matmul per bank per kb)
                        for (o, wd) in wch:
                            nc.tensor.matmul(
                                accW[:, o:o + wd], Ct,
                                Wps[:, b, it * P + o:it * P + o + wd],
                                start=(b == 0), stop=(b == it - 1))

                # ---------- elimination of block it ------------------------- #
                if last:
                    # merged diag block, bordered handling
                    blk7 = fp.tile([P, P], bf, tag="blk7")
                    nc.vector.scalar_tensor_tensor(
                        out=blk7, in0=accW[:, 0:P], scalar=1.0, in1=blk,
                        op0=OP.mult, op1=OP.add)
                    smt = pws.tile([P, n2], dt, tag="ws")
                    rtp = smt[:, 0:1]
                    nc.tensor.matmul(rtp, blk7, eye_bf[:, P - 1:P],
                                     start=True, stop=True)
                    rvec = sp.tile([P, 1], dt, tag="rvec")
                    nc.vector.tensor_scalar_mul(rvec, rtp, -1.0)
                    cvec_sb = sp.tile([P, 1], dt, tag="cvec")
                    nc.vector.tensor_scalar_mul(cvec_sb, blk7[:, P - 1:P], -1.0)
                    nc.vector.tensor_copy(blk7[:, P - 1:P], w127n_col)
                    G = fp.tile([P, P], bf, tag="G")
                    nc.vector.scalar_tensor_tensor(
                        out=G, in0=uu, scalar=c_cols[:, it:it + 1], in1=blk7,
                        op0=OP.mult, op1=OP.subtract)
                elif it == 0:
                    G = Gbase[:, 0, :]
                else:
                    G = fp.tile([P, P], bf, tag="G")
                    with tc.high_priority():
                        nc.vector.scalar_tensor_tensor(
                            out=G, in0=accW[:, 0:P], scalar=-1.0,
                            in1=Gbase[:, it, :], op0=OP.mult, op1=OP.add)
                with tc.high_priority():
                    d = sp.tile([P, 1], dt, tag="d")
                    nc.vector.scalar_tensor_tensor(
                        out=scrB, in0=eye_bf, scalar=1.0, in1=G,
                        op0=OP.mult, op1=OP.mult, accum_out=d)
                    rinv = sp.tile([P, 1], dt, tag="rinv")
                    nc.vector.reciprocal(rinv, d)
                    F = fp.tile([P, P], bf, tag="F")
                    nc.vector.scalar_tensor_tensor(
                        out=F, in0=G, scalar=rinv, in1=eye_bf,
                        op0=OP.mult, op1=OP.subtract)
                    tps = ptr.tile([P, P], bf, tag="tr")
                    nc.tensor.transpose(tps, F, eye_bf)
                lnd = sp.tile([P, 1], dt, tag="lnd")
                nc.scalar.activation(lnd, d, AF.Ln)
                nc.vector.tensor_add(ld_acc, ld_acc, lnd)
                # merged off-diag row MB' = base + acc (fills the transpose
                # wait on DVE); wide tail (it<=3) via a second one-bank pass
                MBv = None
                if not last:
                    m = m_it - P
                    if it == 0:
                        MBv = Mrow[:, 0, P:N]
                    else:
                        MB = eep.tile([P, 768], bf, tag="MB")
                        mA = min(m_it, n2) - P
                        nc.vector.scalar_tensor_tensor(
                            out=MB[:, 0:mA], in0=accW[:, P:P + mA],
                            scalar=1.0,
                            in1=Mrow[:, it, (it + 1) * P:(it + 1) * P + mA],
                            op0=OP.mult, op1=OP.add)
                        if m_it > n2:
                            mB = m_it - n2
                            accB = paw.tile([P, n2], dt, tag="aw")
                            for b2 in range(it):
                                nc.tensor.matmul(
                                    accB[:, 0:mB],
                                    (Att[:, 0, :] if b2 == 0
                                     else Ctt[:, b2, :]),
                                    Wps[:, b2, it * P + n2:N],
                                    start=(b2 == 0), stop=(b2 == it - 1))
                            nc.vector.scalar_tensor_tensor(
                                out=MB[:, mA:m], in0=accB[:, 0:mB],
                                scalar=1.0,
                                in1=Mrow[:, it, it * P + n2:N],
                                op0=OP.mult, op1=OP.add)
                        MBv = MB[:, 0:m]
                Ft = fp.tile([P, P], bf, tag="Ft")
                with tc.high_priority():
                    nc.scalar.copy(Ft, tps)
                t2 = sp.tile([P, 1], dt, tag="t2")
                nc.vector.scalar_tensor_tensor(
                    out=scrB, in0=F, scalar=-0.5, in1=Ft,
                    op0=OP.mult, op1=OP.mult, accum_out=t2)
                nc.vector.tensor_add(ld_acc, ld_acc, t2)

                if not last:
                    # critical W path: order-1 Neumann WITHOUT the rank-1
                    # det-lemma W-correction (logdet keeps its correction
                    # below; the W impact is higher-order and within the
                    # error budget) -> chain is G,d,rinv,F,Ft,PnTFs,W only.
                    with tc.high_priority():
                        PnTF = fp.tile([P, P], bf, tag="PnTF")
                        nc.vector.scalar_tensor_tensor(
                            out=PnTF, in0=Ft, scalar=-1.0, in1=eye_bf,
                            op0=OP.mult, op1=OP.add)
                        PnTFs = fp.tile([P, P], bf, tag="PnTFs")
                        nc.vector.tensor_scalar_mul(PnTFs, PnTF, rinv)
                        m = m_it - P
                        wps = pws.tile([P, n2], dt, tag="ws")
                        nc.tensor.matmul(wps[:, 0:P], PnTFs, MBv[:, 0:P],
                                         start=True, stop=True)
                        nc.scalar.copy(Wps[:, it, (it + 1) * P:
                                           (it + 2) * P], wps[:, 0:P])
                    for (o, wd) in _wchunks(m)[1:]:
                        wps = pws.tile([P, n2], dt, tag="ws")
                        nc.tensor.matmul(wps[:, :wd], PnTFs, MBv[:, o:o + wd],
                                         start=True, stop=True)
                        nc.scalar.copy(Wps[:, it, (it + 1) * P + o:
                                           (it + 1) * P + o + wd],
                                       wps[:, :wd])
                    # det-lemma logdet scalar correction dropped for
                    # interior blocks: the grounding shift contributes
                    # ~1e-4 relative, far under the 2e-2 budget
                else:
                    # bordered last block, root in column 127
                    smt2 = pws.tile([P, n2], dt, tag="ws")
                    F2p = smt2[:, 0:P]
                    nc.tensor.matmul(F2p, F, Ft, start=True, stop=True)
                    PnT2 = fp.tile([P, P], bf, tag="PnT2")
                    nc.vector.scalar_tensor_tensor(
                        out=PnT2, in0=Ft, scalar=-1.0, in1=eye_bf,
                        op0=OP.mult, op1=OP.add)
                    nc.vector.tensor_add(PnT2, PnT2, F2p)
                    x0 = sp.tile([P, 1], dt, tag="x0")
                    nc.vector.tensor_mul(x0, rinv, u_col)
                    x0_bf = sp.tile([P, 1], bf, tag="x0b")
                    nc.vector.tensor_copy(x0_bf, x0)
                    qp = smt2[:, P:P + 1]
                    nc.tensor.matmul(qp, PnT2, x0_bf, start=True, stop=True)
                    qm = sp.tile([P, 1], dt, tag="qm")
                    nc.vector.tensor_mul(qm, qp, u_col)
                    chat = sp.tile([P, 1], dt, tag="chat")
                    nc.vector.tensor_mul(chat, cvec_sb, u_col)
                    x0c = sp.tile([P, 1], dt, tag="x0c")
                    nc.vector.tensor_mul(x0c, rinv, chat)
                    x0c_bf = sp.tile([P, 1], bf, tag="x0cb")
                    nc.vector.tensor_copy(x0c_bf, x0c)
                    y1p = smt2[:, P + 1:P + 2]
                    nc.tensor.matmul(y1p, PnT2, x0c_bf, start=True, stop=True)
                    y1m = sp.tile([P, 1], dt, tag="y1m")
                    nc.vector.tensor_mul(y1m, y1p, u_col)
                    dots = smt2[0:1, 2 * P:2 * P + 5]
                    nc.tensor.matmul(dots[:, 0:1], y1m, ones_col,
                                     start=True, stop=True)
                    nc.tensor.matmul(dots[:, 1:2], rvec, y1m,
                                     start=True, stop=True)
                    nc.tensor.matmul(dots[:, 2:3], rvec, qm,
                                     start=True, stop=True)
                    nc.tensor.matmul(dots[:, 3:4], qm, ones_col,
                                     start=True, stop=True)
                    nc.tensor.matmul(dots[:, 4:5], rvec, eyef[:, P - 1:P],
                                     start=True, stop=True)
                    dsb = sp.tile([1, 5], dt, tag="dsb")
                    nc.vector.tensor_copy(dsb, dots)
                    tac = sp.tile([1, 1], dt, tag="tac")
                    nc.vector.tensor_mul(tac, dsb[:, 3:4],
                                         c_cols[0:1, it:it + 1])
                    detr = sp.tile([1, 1], dt, tag="detr")
                    nc.vector.tensor_scalar(
                        out=detr, in0=tac, scalar1=-1.0, scalar2=1.0,
                        op0=OP.mult, op1=OP.add)
                    lndr = sp.tile([1, 1], dt, tag="lndr")
                    nc.scalar.activation(lndr, detr, AF.Ln)
                    nc.vector.tensor_add(ld_acc[0:1, :], ld_acc[0:1, :], lndr)
                    invdr = sp.tile([1, 1], dt, tag="invdr")
                    nc.vector.reciprocal(invdr, detr)
                    gam = sp.tile([1, 1], dt, tag="gam")
                    nc.vector.tensor_mul(gam, c_cols[0:1, it:it + 1], invdr)
                    bg = sp.tile([1, 1], dt, tag="bg")
                    nc.vector.tensor_mul(bg, dsb[:, 0:1], gam)
                    t3 = sp.tile([1, 1], dt, tag="t3")
                    nc.vector.tensor_mul(t3, bg, dsb[:, 2:3])
                    t4 = sp.tile([1, 1], dt, tag="t4")
                    nc.vector.tensor_sub(t4, dsb[:, 4:5], dsb[:, 1:2])
                    sca = sp.tile([1, 1], dt, tag="sca")
                    nc.vector.tensor_sub(sca, t4, t3)
                    lnsc = sp.tile([1, 1], dt, tag="lnsc")
                    nc.scalar.activation(lnsc, sca, AF.Ln)
                    nc.vector.tensor_add(ld_acc[0:1, :], ld_acc[0:1, :], lnsc)

            # HAM warm-keeper: lowest-priority dummy matmuls that the
            # scheduler slots into PE idle gaps, keeping the activity
            # monitor from re-throttling the PE clock to 1.2 GHz.  Each
            # batch reads its iteration's Mrow tile so late batches cannot
            # be hoisted early (supply survives into the tail).
            for it, ndm in enumerate((8, 5, 6, 6, 8, 8, 9, 10)):
                for _ in range(ndm):
                    nc.tensor.matmul(dmt[:, 0:64], eye_bf, Mrow[:, it, 0:64],
                                     start=True, stop=True)

        # ================= gold bilinear + finale ========================== #
        with (
            tc.tile_pool(name="pf", bufs=1, space=MemorySpace.PSUM) as pf,
            tc.tile_pool(name="fin", bufs=1) as finp,
        ):
            sg = finp.tile([P, 1], dt, tag="sg")
            nc.vector.tensor_add(sg, sacc[:, 0:1], sacc[:, 1:2])
            nc.vector.tensor_add(sg, sg, sacc[:, 2:3])
            nc.vector.tensor_add(sg, sg, sacc[:, 3:4])
            fin = pf.tile([1, 8], dt, tag="fin")
            nc.tensor.matmul(fin[:, 0:1], ld_acc, ones_col, start=True, stop=True)
            nc.tensor.matmul(fin[:, 1:2], sg, ones_col, start=True, stop=True)
            gsum = finp.tile([2, 1], dt, tag="gsum")
            nc.vector.tensor_reduce(gsum, gdots, AX.X, OP.add)
            nc.tensor.matmul(fin[:, 2:3], gsum, ones_col[0:2, :],
                             start=True, stop=True)
            fsb = finp.tile([1, 3], dt, tag="fsb")
            nc.vector.tensor_copy(fsb, fin[:, 0:3])
            f1 = finp.tile([1, 1], dt, tag="f1")
            nc.vector.tensor_sub(f1, fsb[:, 0:1], fsb[:, 1:2])
            f2 = finp.tile([1, 1], dt, tag="f2")
            nc.vector.tensor_sub(f2, f1, fsb[:, 2:3])
            f3 = finp.tile([1, 1], dt, tag="f3")
            nc.vector.tensor_sub(f3, f2, gold_root)
            out_sb = finp.tile([1, 1], dt, tag="out")
            nc.scalar.activation(out_sb, f3, AF.Relu)
            nc.sync.dma_start(loss_d[:, :], out_sb)

    _CACHE["nc"] = nc
    return nc


def finalize_nc(nc):
    """Prepare nc for NEFF compilation (mutates the module; sim-incompatible)."""
    if getattr(nc, "_finalized", False):
        return nc
    from concourse import mybir

    mybir.codegen_inst_isa_subclasses(nc)
    fixed_json = _split_multi_waits(nc.to_json_bytes())
    nc.to_json_bytes = lambda: fixed_json
    nc._finalized = True
    return nc


# --------------------------------------------------------------------------- #
# host-side sharding / prep
# --------------------------------------------------------------------------- #
def _cvec():
    c = np.zeros((P, 3), F32)
    c[:, 0] = 1.0
    c[P - 1, 0] = 0.0
    c[P - 1, 1] = 1.0
    c[P - 1, 2] = -1.0
    return c


def prep_in_maps(inputs):
    import ml_dtypes

    BF = ml_dtypes.bfloat16
    h_cat = np.asarray(inputs["h_cat"], F32)
    left = np.asarray(inputs["left_adj"], F32)
    right = np.asarray(inputs["right_adj"], F32)
    roots = np.asarray(inputs["roots"])
    W_bilin = np.asarray(inputs["W_bilin"], F32)
    b_bilin = np.asarray(inputs["b_bilin"], F32)
    W_head = np.asarray(inputs["W_head"], F32)
    W_dep = np.asarray(inputs["W_dep"], F32)
    W_r1 = np.asarray(inputs["W_r1"], F32)
    b_r1 = np.asarray(inputs["b_r1"], F32)
    W_r2 = np.asarray(inputs["W_r2"], F32)
    b_r2 = np.asarray(inputs["b_r2"], F32)

    # Whd col pairs -> [dep0,0], [0,head0], [dep1,0], [0,head1]
    z = np.zeros(H, F32)
    whd = np.stack([W_dep[0], z, z, W_head[0], W_dep[1], z, z, W_head[1]],
                   axis=1)
    bcb = np.array([[0.0, 1.0, 0.0, 1.0],
                    [1.0, b_bilin[0], 1.0, b_bilin[1]]], F32)
    uu = np.ones((P, P), F32)
    uu[:, P - 1] = 0.0
    uu[P - 1, :] = 0.0
    shared = {
        "W0": np.ascontiguousarray(W_bilin[0]).astype(BF),
        "W1": np.ascontiguousarray(W_bilin[1]).astype(BF),
        "Whd": np.ascontiguousarray(whd).astype(BF),
        "bcb": bcb,
        "uu": uu.astype(BF),
        "Wr1T": np.ascontiguousarray(W_r1.T).astype(BF),
        "br1": np.ascontiguousarray(b_r1.reshape(H, 1)),
        "Wr2T": np.ascontiguousarray(W_r2.reshape(1, H).T).astype(BF),
        "bvec": np.ascontiguousarray(
            np.array([b_bilin[0], b_bilin[1], b_r2.reshape(-1)[0], 0.0],
                     F32).reshape(1, 4)),
        "cvec": _cvec(),
    }
    in_maps = []
    idx = np.arange(N)
    for b in range(B):
        hp = np.roll(h_cat[b], -1, axis=0)
        Lp = np.roll(np.roll(left[b], -1, axis=0), -1, axis=1)
        Rp = np.roll(np.roll(right[b], -1, axis=0), -1, axis=1)
        par = np.argmax(Lp + Rp, axis=0)
        mask0 = Lp[par, idx] > 0
        mask1 = Rp[par, idx] > 0
        Q0 = np.where(mask0[:, None], hp[par], 0).astype(F32)
        Q1 = np.where(mask1[:, None], hp[par], 0).astype(F32)
        # cm pairs: J_k -> [mask_k; 0], I_k -> [0; cnt_k]
        cm = np.zeros((2, 4, N), F32)
        cm[0, 0] = mask0.astype(F32)
        cm[1, 1] = np.bincount(par[mask0], minlength=N)
        cm[0, 2] = mask1.astype(F32)
        cm[1, 3] = np.bincount(par[mask1], minlength=N)
        cm = cm.reshape(2, 4 * N)
        onehot = np.zeros((1, N), F32)
        onehot[0, (int(roots[b]) - 1) % N] = 1.0
        m = dict(shared)
        m["hpT"] = np.ascontiguousarray(hp.T).astype(BF)
        m["hN"] = np.ascontiguousarray(hp).astype(BF)
        m["Q0"] = np.ascontiguousarray(Q0).astype(BF)
        m["Q1"] = np.ascontiguousarray(Q1).astype(BF)
        m["cm"] = np.ascontiguousarray(cm).astype(BF)
        m["onehot"] = onehot
        in_maps.append(m)
    return in_maps


def kernel(**inputs):
    global LAST_RESULTS
    nc = finalize_nc(build_nc())
    in_maps = prep_in_maps(inputs)
    from concourse.bass_utils import run_bass_kernel_spmd

    trace = bool(os.environ.get("KERNEL_TRACE"))
    res = run_bass_kernel_spmd(nc, in_maps, list(range(B)), trace=trace)
    LAST_RESULTS = res
    losses = np.array([res.results[i]["loss"][0, 0] for i in range(B)], F32)
    return np.asarray(F32(ALPHA) * losses.sum(dtype=F32) / F32(B))
